# revision 1
# baseline (speedup 1.0000x reference)
"""Trainium2 Bass/Tile kernel for pre-LN causal multi-head self-attention.

Problem shapes (hardcoded): x (4, 2048, 512), 8 heads, dq=dv=64, fp32.

Sharding over 8 NeuronCores: core c handles batch n = c//2 and the 4 heads
h in [4*(c%2), 4*(c%2)+4).  Every core runs the SAME program (SPMD); all
per-core differences are carried by input values:
  - x:       the core's batch (2048, 512)
  - resid:   even cores: x[n] (residual); odd cores: broadcast bo rows.
             Each term of (residual + bo) is added exactly once per pair.
  - wqkv:    (512, 768) gamma-folded [Wq | Wk | Wv] column slices for the
             core's 4 heads
  - bcol:    (128, 4) q/k projection bias columns (beta @ W + b)
  - bv:      (256,) v-projection bias (zero-specialized when all-zero)
  - wo:      (256, 512) Wo rows for the core's 4 heads
  - pad01:   (128, 16) key-padding mask column per key tile
  - selgrid: (128, 1024) selector matrix for denominator broadcast
Host combines: out[n] = y_part[2n] + y_part[2n+1].

Dataflow (everything stays transposed; all matmuls run as float32r):
  LN:      bn_stats/bn_aggr; rstd = Exp(-0.5*Ln(var+eps)) on ScalarE so the
           whole kernel stays in one ACT table set; gamma/beta folded into
           weights/biases on the host.
  xnT:     PE 128x128 transposes -> xnT [d, s]; evacuated on ScalarE.
  q/k:     qT/kT = W^T @ xnT [128, 2048]; two heads per tile (partition
           halves) enabling row-packed (tile_position) score matmuls.
  v:       v [s, 4, 65] tiles with a ones column; padded key rows zeroed
           (exact key-padding mask: zero contribution to numerator AND
           denominator).
  scores:  sT[key, query] = kT^T @ qT, both heads into one 2-bank psum
           tile -> ONE fused exp [128, 2, 512-o] per key tile (scale=1/8
           folded in); causal masking by zeroing sub-diagonal p on GPSIMD
           (memset + affine_select), diagonal tiles only.
  PV:      out'[65, 512] += v'^T @ p accumulated in psum; row 64 collects
           softmax denominators via the ones column.
  norm:    denominator rows DMA-packed into dcoll (partition-legal bases),
           batched exact reciprocal, K=128 selector matmul broadcasts the
           recip row across partitions, one in-place DVE multiply.
  outproj: y = outTP0^T@Wo01 + h2/h3 unpaired + residual add on DVE.

Schedule: the main loop interleaves projection chunk m with attention
blocks m for both head-pairs (attention m needs only chunks <= m), keeping
the exp-bound ScalarE fed; m<=2 normalize + output projection are dripped
into block 3's attention loop; only chunk 3's normalize + outproj remain
in the tail.  One shared PSUM pool budgets exactly 8 banks:
pa(2) + s(2x2) + po0(1) + po1(1), with rb->pa and psy->s tag reuse.
"""

import numpy as np

S = 2048
D = 512
DQ = 64
H_PER_CORE = 4
N_CORES = 8
LN_EPS = 1e-5
NEG = -30.0

_PROGRAM = {}


def _install_tile_patch():
    """Workarounds for walrus/concourse skew in this container:

    1. This walrus build rejects instructions carrying more than one
       semaphore-wait command ("Too many sync wait commands"), but Tile's
       rust wait-assigner freely emits 2-3 waits per instruction.  After
       wait assignment, split excess waits onto EventSemaphore carrier
       instructions inserted just before the owner on the same engine.
    2. Tile's tail drain carries one wait per outstanding proc; split into
       one drain per proc.
    3. Tile's tail emits a gpsimd sem_clear (Pool ISA opcode 176) that this
       walrus rejects ("ISA wrong length").  The NRT preamble's sema_reset
       zeroes user semaphores at every execution, so the tail clear is
       redundant and skipped.
    """
    from bass_rust import SyncInfo as _SyncInfo

    from concourse import mybir, tile
    from concourse.vector_clock import ScopedClock, VectorClock

    if getattr(tile.TileContext, "_mha_patch", False):
        return

    MAXW = 1

    def _split_excess_waits(ordered, nc):
        for bb_name, insts in list(ordered.items()):
            out = []
            changed = False
            for inst in insts:
                si = inst.sync_info
                if si is None:
                    out.append(inst)
                    continue
                waits = list(si.on_wait)
                if len(waits) > MAXW:
                    changed = True
                    excess = waits[:-MAXW]
                    for k in range(0, len(excess), MAXW):
                        carrier = mybir.InstEventSemaphore(
                            name=f"wsplit-{nc.next_id()}"
                        )
                        carrier.engine = inst.engine
                        carrier.bass_scheduled_proc = inst.bass_scheduled_proc
                        carrier.bass_scheduled_scope = inst.bass_scheduled_scope
                        carrier.bass_scheduled_tick = inst.bass_scheduled_tick
                        carrier.sync_info = _SyncInfo(
                            on_wait=excess[k : k + MAXW], on_update=[]
                        )
                        out.append(carrier)
                    si.on_wait = waits[-MAXW:]
                    inst.sync_info = si
                out.append(inst)
            if changed:
                ordered[bb_name] = out

    _RustTileClockWait = tile.TileClockWait

    class _SplittingTileClockWait:
        def __init__(self, tc, ordered, **kw):
            self._inner = _RustTileClockWait(tc, ordered, **kw)
            self._ordered = ordered
            self._nc = tc.nc

        def __getattr__(self, k):
            return getattr(self._inner, k)

        def assign_waits(self, bb_name):
            self._inner.assign_waits(bb_name)
            _split_excess_waits(self._ordered, self._nc)

    tile.TileClockWait = _SplittingTileClockWait

    def _patched_drain_and_barrier(self, tick_clock, wait_clock):
        gvec = tick_clock.global_clock
        n = len(gvec)
        for i in range(n):
            if gvec[i] > 0:
                v = VectorClock([gvec[j] if j == i else 0 for j in range(n)])
                d = self.nc.sync.drain()
                wait_clock.add_sem_waits(d.ins, ScopedClock({None: v}))
        self.nc.all_engine_barrier()
        assert self.sems is not None
        popped = self.nc._tile_sem_poison_stack.pop()
        assert popped is self._sem_poison
        self.nc.all_engine_barrier()

    tile.TileContext._drain_and_barrier = _patched_drain_and_barrier

    # cayman has 208 KiB usable per partition; the stale 192 KiB constant
    # leaves 16 KiB on the table
    from concourse import tile_utils

    tile_utils.max_sbuf_usage = 208 * 1024
    tile.TileContext._mha_patch = True


def _build_program(bv_zero=False):
    _BV_ZERO = bv_zero
    from contextlib import ExitStack

    import concourse.bass as bass
    import concourse.tile as tile
    from concourse import mybir
    from concourse.masks import make_identity

    f32 = mybir.dt.float32
    f32r = mybir.dt.float32r
    AF = mybir.ActivationFunctionType

    nc = bass.Bass()

    x_d = nc.dram_tensor("x", [S, D], f32, kind="ExternalInput")
    resid_d = nc.dram_tensor("resid", [S, D], f32, kind="ExternalInput")
    wqkv_d = nc.dram_tensor("wqkv", [D, 768], f32r, kind="ExternalInput")
    bcol_d = nc.dram_tensor("bcol", [128, 4], f32, kind="ExternalInput")
    bv_d = nc.dram_tensor("bv", [256], f32, kind="ExternalInput")
    wo_d = nc.dram_tensor("wo", [256, D], f32r, kind="ExternalInput")
    pad_d = nc.dram_tensor("pad01", [128, 16], f32, kind="ExternalInput")
    sel_d = nc.dram_tensor("selgrid", [128, 1024], f32r, kind="ExternalInput")
    y_d = nc.dram_tensor("y", [S, D], f32, kind="ExternalOutput")

    _install_tile_patch()

    with tile.TileContext(nc) as tc, ExitStack() as ctx:
        consts = ctx.enter_context(tc.tile_pool(name="consts", bufs=1))
        big = ctx.enter_context(tc.tile_pool(name="big", bufs=1))
        # one shared PSUM pool; tag budget adds up to exactly 8 banks so all
        # phases can be in flight at once:
        #   pa(2) + sA(2) + sB(2) + poA(1) + poB(1) = 8
        # (rb reuses sA slots, psum_y reuses sB slots later in the kernel)
        ps = ctx.enter_context(tc.tile_pool(name="ps", bufs=2, space="PSUM"))
        xa_pool = ctx.enter_context(tc.tile_pool(name="xa", bufs=4))
        xn_pool = ctx.enter_context(tc.tile_pool(name="xn", bufs=6))
        st_pool = ctx.enter_context(tc.tile_pool(name="st", bufs=4))
        p_pool = ctx.enter_context(tc.tile_pool(name="pp", bufs=4))
        dt_pool = ctx.enter_context(tc.tile_pool(name="dt", bufs=4))
        r_pool = ctx.enter_context(tc.tile_pool(name="rr", bufs=4))
        y_pool = ctx.enter_context(tc.tile_pool(name="yy", bufs=4))

        # ---- constants and weights ----
        ident = consts.tile([128, 128], f32, tag="ident")
        make_identity(nc, ident)
        eps_col = consts.tile([128, 1], f32, tag="eps")
        nc.vector.memset(eps_col, LN_EPS)
        bcol = consts.tile([128, 4], f32, tag="bcol")
        nc.sync.dma_start(out=bcol, in_=bcol_d[:, :])
        pad01 = consts.tile([128, 16], f32, tag="pad01")
        nc.sync.dma_start(out=pad01, in_=pad_d[:, :])
        selgrid = consts.tile([128, 16 * 64], f32r, tag="selgrid")
        wo_pair0 = big.tile([128, D], f32r, tag="wop0", name="wop0")
        wo_h2 = big.tile([64, D], f32r, tag="woh2", name="woh2")
        wo_h3 = big.tile([64, D], f32r, tag="woh3", name="woh3")
        bv_bc = consts.tile([128, 256], f32, tag="bv")
        nc.sync.dma_start(out=bv_bc, in_=bv_d[None, :].to_broadcast([128, 256]))

        wqkv_r = wqkv_d.rearrange("(a p) j -> a p j", p=128)
        wqkv_sb = [
            big.tile([128, 768], f32r, tag=f"wqkv{db}", name=f"wqkv{db}")
            for db in range(4)
        ]

        # persistent big tensors
        xnT = [big.tile([128, S], f32r, tag=f"xnT{db}", name=f"xnT{db}") for db in range(4)]
        qT = [big.tile([128, S], f32r, tag=f"qT{hp}", name=f"qT{hp}") for hp in range(2)]
        kT = [big.tile([128, S], f32r, tag=f"kT{hp}", name=f"kT{hp}") for hp in range(2)]
        v_sb = [big.tile([128, 4, 65], f32r, tag=f"v{st}", name=f"vsb{st}") for st in range(16)]
        # normalized attention outputs, paired: outTP[hp] rows 0:64 = head
        # 2hp (written directly), rows 64:128 = head 2hp+1 (DMA'd from outTo)
        outTP = [big.tile([128, S], f32r, tag=f"outTP{hp}", name=f"outTP{hp}") for hp in range(2)]
        outTo = [big.tile([64, S], f32r, tag=f"outTo{hp}", name=f"outTo{hp}") for hp in range(2)]
        dcoll = big.tile([128, 512], f32, tag="dcoll", name="dcoll")
        dcoll_r0 = big.tile([128, 512], f32, tag="dcollr0", name="dcollr0")
        dcoll_r = big.tile([128, 512], f32r, tag="dcollr", name="dcollr")
        nc.vector.memset(dcoll_r.bitcast(f32), 1.0)

        # ==== interleaved main loop: chunk blk of LN/proj, then the two
        # head-pairs' attention m=blk blocks (which need only chunks <= blk).
        # This keeps ScalarE (exp-bound) fed while PE does projections. ====
        from collections import deque

        pending_work = deque()
        for blk in range(4):
            chv = blk
            # ---- phase A work for chunk blk ----
            xn_tiles = []
            for ss in range(4):
                stv = chv * 4 + ss
                x_t = xa_pool.tile([128, D], f32, tag="x", name="xt")
                nc.sync.dma_start(out=x_t, in_=x_d[stv * 128 : (stv + 1) * 128, :])
                stats = st_pool.tile([128, 6], f32, tag="stats", name="stats")
                nc.vector.bn_stats(out=stats, in_=x_t)
                mv = st_pool.tile([128, 2], f32, tag="mv", name="mv")
                nc.vector.bn_aggr(out=mv, in_=stats)
                lnv = st_pool.tile([128, 1], f32, tag="lnv", name="lnv")
                nc.scalar.activation(
                    out=lnv, in_=mv[:, 1:2], func=AF.Ln, bias=eps_col, scale=1.0
                )
                rstd = st_pool.tile([128, 1], f32, tag="rstd", name="rstd")
                nc.scalar.activation(
                    out=rstd, in_=lnv, func=AF.Exp, bias=0.0, scale=-0.5
                )
                xn_t = xn_pool.tile([128, D], f32, tag="xn", name="xnt")
                nc.vector.tensor_scalar(
                    out=xn_t,
                    in0=x_t,
                    scalar1=mv[:, 0:1],
                    scalar2=rstd,
                    op0=mybir.AluOpType.subtract,
                    op1=mybir.AluOpType.mult,
                )
                xn_tiles.append(xn_t)

            if chv == 0:
                for db in range(4):
                    nc.sync.dma_start(out=wqkv_sb[db], in_=wqkv_r[db])

            # transpose 4x4 128x128 blocks: xn [s,d] -> xnT [d,s].
            # chunk 0 goes per-s-tile so PE starts right after the first
            # LN tile instead of waiting for all four (shorter lead-in).
            if chv == 0:
                for ss in range(4):
                    ps_t = ps.tile([128, D], f32, tag="pa", name="pst")
                    for db in range(4):
                        nc.tensor.transpose(
                            out=ps_t[:, db * 128 : (db + 1) * 128],
                            in_=xn_tiles[ss][:, db * 128 : (db + 1) * 128],
                            identity=ident,
                        )
                    for db in range(4):
                        nc.scalar.copy(
                            out=xnT[db][:, ss * 128 : (ss + 1) * 128],
                            in_=ps_t[:, db * 128 : (db + 1) * 128],
                        )
            else:
                for db in range(4):
                    ps_t = ps.tile([128, D], f32, tag="pa", name="pst")
                    for ss in range(4):
                        nc.tensor.transpose(
                            out=ps_t[:, ss * 128 : (ss + 1) * 128],
                            in_=xn_tiles[ss][:, db * 128 : (db + 1) * 128],
                            identity=ident,
                        )
                    nc.scalar.copy(
                        out=xnT[db][:, chv * 512 : (chv + 1) * 512], in_=ps_t
                    )

            # q/k projections: 4 j-tiles (q-pair0, q-pair1, k-pair0, k-pair1)
            for jt in range(4):
                dst = qT[jt] if jt < 2 else kT[jt - 2]
                ps_qk = ps.tile([128, 512], f32, tag="pa", name="psqk")
                for db in range(4):
                    nc.tensor.matmul(
                        out=ps_qk,
                        lhsT=wqkv_sb[db][:, jt * 128 : (jt + 1) * 128],
                        rhs=xnT[db][:, chv * 512 : (chv + 1) * 512],
                        start=(db == 0),
                        stop=(db == 3),
                    )
                nc.vector.tensor_scalar_add(
                    out=dst[:, chv * 512 : (chv + 1) * 512],
                    in0=ps_qk,
                    scalar1=bcol[:, jt : jt + 1],
                )

            # v projection: [s, e] orientation with ones column + padding
            for ss in range(4):
                stv = chv * 4 + ss
                ps_v = ps.tile([128, 256], f32, tag="pa", name="psv")
                for db in range(4):
                    nc.tensor.matmul(
                        out=ps_v,
                        lhsT=xnT[db][:, stv * 128 : (stv + 1) * 128],
                        rhs=wqkv_sb[db][:, 512:768],
                        start=(db == 0),
                        stop=(db == 3),
                    )
                vt = v_sb[stv]
                nc.gpsimd.memset(vt.bitcast(f32)[:, :, 64:65], 1.0)
                if _BV_ZERO:
                    # (psum + 0) * pad in one op; the ones column is scaled
                    # by a separate tiny op
                    nc.vector.tensor_scalar_mul(
                        out=vt[:, :, 0:64],
                        in0=ps_v.rearrange("p (h e) -> p h e", h=4),
                        scalar1=pad01[:, stv : stv + 1],
                    )
                    nc.vector.tensor_scalar_mul(
                        out=vt[:, :, 64:65],
                        in0=vt[:, :, 64:65],
                        scalar1=pad01[:, stv : stv + 1],
                    )
                else:
                    nc.vector.tensor_tensor(
                        out=vt[:, :, 0:64],
                        in0=ps_v.rearrange("p (h e) -> p h e", h=4),
                        in1=bv_bc.rearrange("p (h e) -> p h e", h=4),
                        op=mybir.AluOpType.add,
                    )
                    nc.vector.tensor_scalar_mul(
                        out=vt[:, :, :],
                        in0=vt[:, :, :],
                        scalar1=pad01[:, stv : stv + 1],
                    )

            if chv == 0:
                # late-loaded weights (needed at normalization / outproj)
                nc.sync.dma_start(out=selgrid, in_=sel_d[:, :])
                nc.sync.dma_start(out=wo_pair0, in_=wo_d[0:128, :])
                nc.sync.dma_start(out=wo_h2, in_=wo_d[128:192, :])
                nc.sync.dma_start(out=wo_h3, in_=wo_d[192:256, :])

            # ---- attention m = blk for both head-pairs ----
            m = blk
            for hp in range(2):
                po = {}
                for ab in range(2):
                    po[ab] = ps.tile(
                        [65, 512], f32, tag=f"po{ab}", bufs=1, name=f"po{ab}"
                    )
                njt = 4 * m + 4  # key tiles 0 .. 4m+3
                for j in range(njt):
                    diag_o = 128 * (j - 4 * m) if j >= 4 * m else None
                    o = diag_o if diag_o is not None else 0
                    # both heads' scores in one 2-bank tile -> one fused exp
                    ps_s = ps.tile([128, 2, 512], f32, tag="s", name="pss")
                    for ab in range(2):
                        base = ab * 64
                        nc.tensor.matmul(
                            out=ps_s[:, ab, :],
                            lhsT=kT[hp][base : base + 64, j * 128 : (j + 1) * 128],
                            rhs=qT[hp][base : base + 64, m * 512 : (m + 1) * 512],
                            start=True,
                            stop=True,
                        )
                    p_t = p_pool.tile([128, 2, 512], f32r, tag="p", name="pt")
                    nc.scalar.activation(
                        out=p_t[:, :, o:512],
                        in_=ps_s[:, :, o:512],
                        func=AF.Exp,
                        bias=0.0,
                        scale=0.125,
                    )
                    if diag_o is not None:
                        if o > 0:
                            nc.gpsimd.memset(p_t.bitcast(f32)[:, :, 0:o], 0.0)
                        # zero strictly-below-diagonal: keep col-part>=0
                        nc.gpsimd.affine_select(
                            out=p_t[:, :, o : o + 128],
                            in_=p_t[:, :, o : o + 128],
                            compare_op=mybir.AluOpType.is_ge,
                            fill=0.0,
                            base=0,
                            pattern=[[0, 2], [1, 128]],
                            channel_multiplier=-1,
                        )
                    for ab in range(2):
                        h = hp * 2 + ab
                        nc.tensor.matmul(
                            out=po[ab],
                            lhsT=v_sb[j][:, h, 0:65],
                            rhs=p_t[:, ab, :],
                            start=(j == 0),
                            stop=(j == njt - 1),
                        )
                    if blk == 3 and pending_work:
                        fn, args = pending_work.popleft()
                        fn(*args)
                # evacuate raw numerators + denominator rows (denominator
                # sits at psum partition 64; engines can't shift partitions,
                # so stage at partition 64 and DMA-pack into dcoll)
                for ab in range(2):
                    dst = outTP[hp][0:64, :] if ab == 0 else outTo[hp]
                    nc.vector.tensor_copy(
                        out=dst[:, m * 512 : (m + 1) * 512],
                        in_=po[ab][0:64, :],
                    )
                    d_st = dt_pool.tile([65, 512], f32, tag="dst", name="dst")
                    nc.scalar.copy(out=d_st[64:65, :], in_=po[ab][64:65, :])
                    r = (6 * hp + 3 * ab + m) if m < 3 else (32 * (hp + 1) + ab)
                    nc.sync.dma_start(
                        out=dcoll[r : r + 1, :], in_=d_st[64:65, :]
                    )

                def norm_row(bk_h, mm):
                    # rb = broadcast of recip row; multiply into outTP/outTo
                    hq, abq = divmod(bk_h, 2)
                    bk = bk_h * 4 + mm
                    rb = ps.tile([64, 512], f32, tag="pa", name="rb")
                    nc.tensor.matmul(
                        out=rb,
                        lhsT=selgrid[:, bk * 64 : (bk + 1) * 64],
                        rhs=dcoll_r,
                        start=True,
                        stop=True,
                    )
                    dstq = outTP[hq][0:64, :] if abq == 0 else outTo[hq]
                    nc.vector.tensor_tensor(
                        out=dstq[:, mm * 512 : (mm + 1) * 512],
                        in0=dstq[:, mm * 512 : (mm + 1) * 512],
                        in1=rb,
                        op=mybir.AluOpType.mult,
                    )

                def move0(mm):
                    nc.sync.dma_start(
                        out=outTP[0][64:128, mm * 512 : (mm + 1) * 512],
                        in_=outTo[0][:, mm * 512 : (mm + 1) * 512],
                    )

                def outproj_tile(stv):
                    if True:
                        ps_y = ps.tile([128, 512], f32, tag="s", name="psy")
                        r_t = r_pool.tile([128, D], f32, tag="r", name="rt")
                        nc.sync.dma_start(
                            out=r_t,
                            in_=resid_d[stv * 128 : (stv + 1) * 128, :],
                        )
                        nc.tensor.matmul(
                            out=ps_y,
                            lhsT=outTP[0][:, stv * 128 : (stv + 1) * 128],
                            rhs=wo_pair0[:, :],
                            start=True,
                            stop=False,
                        )
                        nc.tensor.matmul(
                            out=ps_y,
                            lhsT=outTP[1][0:64, stv * 128 : (stv + 1) * 128],
                            rhs=wo_h2[:, :],
                            start=False,
                            stop=False,
                        )
                        nc.tensor.matmul(
                            out=ps_y,
                            lhsT=outTo[1][:, stv * 128 : (stv + 1) * 128],
                            rhs=wo_h3[:, :],
                            start=False,
                            stop=True,
                        )
                        y_t = y_pool.tile([128, D], f32, tag="y", name="yt")
                        nc.vector.tensor_tensor(
                            out=y_t, in0=ps_y, in1=r_t,
                            op=mybir.AluOpType.add,
                        )
                        nc.sync.dma_start(
                            out=y_d[stv * 128 : (stv + 1) * 128, :], in_=y_t
                        )

                if blk == 2 and hp == 1:
                    # all m<=2 denominators (both head-pairs) are in;
                    # reciprocal them now, then drip the normalize +
                    # outproj work into blk 3's attention loop (see
                    # pending_work) so it fills PE/DVE gaps there
                    nc.vector.reciprocal(
                        out=dcoll_r0[0:12, :], in_=dcoll[0:12, :]
                    )
                    nc.vector.tensor_copy(
                        out=dcoll_r[0:12, :], in_=dcoll_r0[0:12, :]
                    )
                    for mm in range(3):
                        pending_work.append((norm_row, (0, mm)))
                        pending_work.append((norm_row, (1, mm)))
                        pending_work.append((move0, (mm,)))
                        pending_work.append((norm_row, (2, mm)))
                        pending_work.append((norm_row, (3, mm)))
                        pending_work.append((outproj_tile, (mm * 4 + 0,)))
                        pending_work.append((outproj_tile, (mm * 4 + 1,)))
                        pending_work.append((outproj_tile, (mm * 4 + 2,)))
                        pending_work.append((outproj_tile, (mm * 4 + 3,)))

                if blk == 3:
                    while pending_work:
                        fn, args = pending_work.popleft()
                        fn(*args)
                    # m=3 denominators per head-pair, at legal bases 32/64
                    lo = 32 * (hp + 1)
                    nc.vector.reciprocal(
                        out=dcoll_r0[lo : lo + 2, :], in_=dcoll[lo : lo + 2, :]
                    )
                    nc.vector.tensor_copy(
                        out=dcoll_r[lo : lo + 2, :], in_=dcoll_r0[lo : lo + 2, :]
                    )
                    norm_row(hp * 2 + 0, 3)
                    norm_row(hp * 2 + 1, 3)
                    if hp == 0:
                        nc.sync.dma_start(
                            out=outTP[0][64:128, 3 * 512 : 4 * 512],
                            in_=outTo[0][:, 3 * 512 : 4 * 512],
                        )
                    else:
                        for ss in range(4):
                            outproj_tile(12 + ss)

    return nc


def _get_program(bv_zero=False):
    if bv_zero not in _PROGRAM:
        _PROGRAM[bv_zero] = _build_program(bv_zero)
    return _PROGRAM[bv_zero]


def _make_in_maps(inputs):
    x = np.ascontiguousarray(np.asarray(inputs["x"], dtype=np.float32))
    lengths = np.asarray(inputs["key_value_sequence_lengths"]).astype(np.int64)
    Wq = np.asarray(inputs["Wq"], dtype=np.float32)
    bq = np.asarray(inputs["bq"], dtype=np.float32)
    Wkv = np.asarray(inputs["Wkv"], dtype=np.float32)
    bkv = np.asarray(inputs["bkv"], dtype=np.float32)
    Wo = np.asarray(inputs["Wo"], dtype=np.float32)
    bo = np.asarray(inputs["bo"], dtype=np.float32)
    gamma = np.asarray(inputs["gamma"], dtype=np.float32)
    beta = np.asarray(inputs["beta"], dtype=np.float32)

    H = 8
    Wk = Wkv[:, : H * DQ]
    Wv = Wkv[:, H * DQ :]
    bk = bkv[: H * DQ]
    bv_full = bkv[H * DQ :]

    in_maps = []
    for c in range(N_CORES):
        n = c // 2
        h0 = 4 * (c % 2)
        hsel = slice(h0 * DQ, (h0 + 4) * DQ)  # 256 contiguous columns

        wq_s = Wq[:, hsel]
        wk_s = Wk[:, hsel]
        wv_s = Wv[:, hsel]
        wqkv = np.concatenate(
            [gamma[:, None] * wq_s, gamma[:, None] * wk_s, gamma[:, None] * wv_s],
            axis=1,
        ).astype(np.float32)
        bq_eff = beta @ wq_s + bq[hsel]
        bk_eff = beta @ wk_s + bk[hsel]
        bv_eff = beta @ wv_s + bv_full[hsel]
        bcol = np.concatenate([bq_eff, bk_eff]).reshape(4, 128).T.copy()
        wo_s = Wo[hsel, :].astype(np.float32)

        ln = int(lengths[n])
        b_idx = np.arange(128)[:, None]
        j_idx = np.arange(16)[None, :]
        pad01 = ((128 * j_idx + b_idx) < ln).astype(np.float32)

        resid = x[n] if c % 2 == 0 else np.ascontiguousarray(
            np.broadcast_to(bo, (S, D)).astype(np.float32)
        )

        sel = np.zeros((128, 1024), np.float32)
        for h in range(4):
            hp_, ab_ = divmod(h, 2)
            for m in range(4):
                blk = h * 4 + m
                if m < 3:
                    row = 6 * hp_ + 3 * ab_ + m
                else:
                    row = 32 * (hp_ + 1) + ab_
                sel[row, blk * 64 : (blk + 1) * 64] = 1.0

        in_maps.append(
            {
                "x": x[n],
                "selgrid": sel,
                "resid": resid,
                "wqkv": wqkv,
                "bcol": np.ascontiguousarray(bcol, dtype=np.float32),
                "bv": bv_eff.astype(np.float32),
                "wo": wo_s,
                "pad01": np.ascontiguousarray(pad01, dtype=np.float32),
            }
        )
    return in_maps


def kernel_run(inputs, trace=False):
    from concourse.bass_utils import run_bass_kernel_spmd

    in_maps = _make_in_maps(inputs)
    bv_zero = all(
        not np.any(np.asarray(m["bv"], dtype=np.float32)) for m in in_maps
    )
    nc = _get_program(bv_zero)
    res = run_bass_kernel_spmd(nc, in_maps, list(range(N_CORES)), trace=trace)
    parts = [res.results[c]["y"] for c in range(N_CORES)]
    out = np.stack(
        [parts[2 * n] + parts[2 * n + 1] for n in range(4)], axis=0
    ).astype(np.float32)
    return out, res


def kernel(**inputs) -> np.ndarray:
    out, _ = kernel_run(inputs)
    return out



# revision 53
# speedup vs baseline: 1.1630x; 1.1630x over previous
"""Trainium2 Bass/Tile kernel for pre-LN causal multi-head self-attention.

Problem shapes (hardcoded): x (4, 2048, 512), 8 heads, dq=dv=64, fp32.

Sharding over 8 NeuronCores: core c handles batch n = c//2 and the 4 heads
h in [4*(c%2), 4*(c%2)+4).  Every core runs the SAME program (SPMD); all
per-core differences are carried by input values:
  - x:       the core's batch (2048, 512)
  - resid:   even cores: x[n] (residual); odd cores: broadcast bo rows.
             Each term of (residual + bo) is added exactly once per pair.
  - wqkv:    (512, 768) gamma-folded [Wq | Wk | Wv] column slices for the
             core's 4 heads
  - bcol:    (128, 4) q/k projection bias columns (beta @ W + b)
  - bv:      (256,) v-projection bias (zero-specialized when all-zero)
  - wo:      (256, 512) Wo rows for the core's 4 heads
  - pad01:   (128, 16) key-padding mask column per key tile
  - selgrid: (128, 1024) selector matrix for denominator broadcast
Host combines: out[n] = y_part[2n] + y_part[2n+1].

Dataflow (everything stays transposed; all matmuls run as float32r):
  LN:      bn_stats/bn_aggr; rstd = Exp(-0.5*Ln(var+eps)) on ScalarE so the
           whole kernel stays in one ACT table set; gamma/beta folded into
           weights/biases on the host.
  xnT:     PE 128x128 transposes -> xnT [d, s]; evacuated on ScalarE.
  q/k:     qT/kT = W^T @ xnT [128, 2048]; two heads per tile (partition
           halves) enabling row-packed (tile_position) score matmuls.
  v:       v [s, 4, 65] tiles with a ones column; padded key rows zeroed
           (exact key-padding mask: zero contribution to numerator AND
           denominator).
  scores:  sT[key, query] = kT^T @ qT, both heads into one 2-bank psum
           tile -> ONE fused exp [128, 2, 512-o] per key tile (scale=1/8
           folded in); causal masking by zeroing sub-diagonal p on GPSIMD
           (memset + affine_select), diagonal tiles only.
  PV:      out'[65, 512] += v'^T @ p accumulated in psum; row 64 collects
           softmax denominators via the ones column.
  norm:    denominator rows DMA-packed into dcoll (partition-legal bases),
           batched exact reciprocal, K=128 selector matmul broadcasts the
           recip row across partitions, one in-place DVE multiply.
  outproj: y = outTP0^T@Wo01 + h2/h3 unpaired + residual add on DVE.

Schedule: the main loop interleaves projection chunk m with attention
blocks m for both head-pairs (attention m needs only chunks <= m), keeping
the exp-bound ScalarE fed; m<=2 normalize + output projection are dripped
into block 3's attention loop; only chunk 3's normalize + outproj remain
in the tail.  One shared PSUM pool budgets exactly 8 banks:
pa(2) + s(2x2) + po0(1) + po1(1), with rb->pa and psy->s tag reuse.
"""

import numpy as np

S = 2048
D = 512
DQ = 64
H_PER_CORE = 4
N_CORES = 8
LN_EPS = 1e-5
NEG = -30.0

_PROGRAM = {}


def _install_tile_patch():
    """Workarounds for walrus/concourse skew in this container:

    1. This walrus build rejects instructions carrying more than one
       semaphore-wait command ("Too many sync wait commands"), but Tile's
       rust wait-assigner freely emits 2-3 waits per instruction.  After
       wait assignment, split excess waits onto EventSemaphore carrier
       instructions inserted just before the owner on the same engine.
    2. Tile's tail drain carries one wait per outstanding proc; split into
       one drain per proc.
    3. Tile's tail emits a gpsimd sem_clear (Pool ISA opcode 176) that this
       walrus rejects ("ISA wrong length").  The NRT preamble's sema_reset
       zeroes user semaphores at every execution, so the tail clear is
       redundant and skipped.
    """
    from bass_rust import SyncInfo as _SyncInfo

    from concourse import mybir, tile
    from concourse.vector_clock import ScopedClock, VectorClock

    if getattr(tile.TileContext, "_mha_patch", False):
        return

    MAXW = 1

    def _split_excess_waits(ordered, nc):
        for bb_name, insts in list(ordered.items()):
            out = []
            changed = False
            for inst in insts:
                si = inst.sync_info
                if si is None:
                    out.append(inst)
                    continue
                waits = list(si.on_wait)
                if len(waits) > MAXW:
                    changed = True
                    excess = waits[:-MAXW]
                    for k in range(0, len(excess), MAXW):
                        carrier = mybir.InstEventSemaphore(
                            name=f"wsplit-{nc.next_id()}"
                        )
                        carrier.engine = inst.engine
                        carrier.bass_scheduled_proc = inst.bass_scheduled_proc
                        carrier.bass_scheduled_scope = inst.bass_scheduled_scope
                        carrier.bass_scheduled_tick = inst.bass_scheduled_tick
                        carrier.sync_info = _SyncInfo(
                            on_wait=excess[k : k + MAXW], on_update=[]
                        )
                        out.append(carrier)
                    si.on_wait = waits[-MAXW:]
                    inst.sync_info = si
                out.append(inst)
            if changed:
                ordered[bb_name] = out

    _RustTileClockWait = tile.TileClockWait

    class _SplittingTileClockWait:
        def __init__(self, tc, ordered, **kw):
            self._inner = _RustTileClockWait(tc, ordered, **kw)
            self._ordered = ordered
            self._nc = tc.nc

        def __getattr__(self, k):
            return getattr(self._inner, k)

        def assign_waits(self, bb_name):
            self._inner.assign_waits(bb_name)
            _split_excess_waits(self._ordered, self._nc)

    tile.TileClockWait = _SplittingTileClockWait

    def _patched_drain_and_barrier(self, tick_clock, wait_clock):
        gvec = tick_clock.global_clock
        n = len(gvec)
        for i in range(n):
            if gvec[i] > 0:
                v = VectorClock([gvec[j] if j == i else 0 for j in range(n)])
                d = self.nc.sync.drain()
                wait_clock.add_sem_waits(d.ins, ScopedClock({None: v}))
        self.nc.all_engine_barrier()
        assert self.sems is not None
        popped = self.nc._tile_sem_poison_stack.pop()
        assert popped is self._sem_poison
        self.nc.all_engine_barrier()

    tile.TileContext._drain_and_barrier = _patched_drain_and_barrier

    # cayman has 208 KiB usable per partition; the stale 192 KiB constant
    # leaves 16 KiB on the table
    from concourse import tile_utils

    tile_utils.max_sbuf_usage = 208 * 1024
    tile.TileContext._mha_patch = True


def _build_program(bv_zero=False):
    _BV_ZERO = bv_zero
    from contextlib import ExitStack

    import concourse.bass as bass
    import concourse.tile as tile
    from concourse import mybir
    from concourse.masks import make_identity

    f32 = mybir.dt.float32
    f32r = mybir.dt.float32r
    bf16 = mybir.dt.bfloat16
    fp8 = mybir.dt.float8e4
    DR = mybir.MatmulPerfMode.DoubleRow
    AF = mybir.ActivationFunctionType

    nc = bass.Bass()

    x_d = nc.dram_tensor("x", [S, D], f32, kind="ExternalInput")
    resid_d = nc.dram_tensor("resid", [S, D], f32, kind="ExternalInput")
    wqkv_d = nc.dram_tensor(
        "wqkv", [128, 4, 768], mybir.dt.float8e4, kind="ExternalInput"
    )
    bcol_d = nc.dram_tensor("bcol", [128, 4], f32, kind="ExternalInput")
    bv_d = nc.dram_tensor("bv", [256], f32, kind="ExternalInput")
    wo_d = nc.dram_tensor("wo", [64, 4, D], mybir.dt.float8e4, kind="ExternalInput")
    pad_d = nc.dram_tensor("pad01", [128, 16], f32, kind="ExternalInput")
    y_d = nc.dram_tensor("y", [S, D], f32, kind="ExternalOutput")

    _install_tile_patch()

    with tile.TileContext(nc) as tc, ExitStack() as ctx:
        consts = ctx.enter_context(tc.tile_pool(name="consts", bufs=1))
        big = ctx.enter_context(tc.tile_pool(name="big", bufs=1))
        # one shared PSUM pool; tag budget adds up to exactly 8 banks so all
        # phases can be in flight at once:
        #   pa(2) + sA(2) + sB(2) + poA(1) + poB(1) = 8
        # (rb reuses sA slots, psum_y reuses sB slots later in the kernel)
        ps = ctx.enter_context(tc.tile_pool(name="ps", bufs=2, space="PSUM"))
        xa_pool = ctx.enter_context(tc.tile_pool(name="xa", bufs=4))
        xn_pool = ctx.enter_context(tc.tile_pool(name="xn", bufs=6))
        st_pool = ctx.enter_context(tc.tile_pool(name="st", bufs=4))
        p_pool = ctx.enter_context(tc.tile_pool(name="pp", bufs=3))
        dt_pool = ctx.enter_context(tc.tile_pool(name="dt", bufs=4))
        r_pool = ctx.enter_context(tc.tile_pool(name="rr", bufs=2))

        # ---- constants and weights (x-chunk0 DMAs are issued FIRST in the
        # blk-0 body; all const DMAs are emitted after them so the LN start
        # isn't queued behind them on the single HWDGE ring) ----
        ident = consts.tile([128, 128], bf16, tag="ident")
        eps_col = consts.tile([128, 1], f32, tag="eps")
        nc.vector.memset(eps_col, LN_EPS)
        bcol = consts.tile([128, 4], f32, tag="bcol")
        pad01 = consts.tile([128, 16], f32, tag="pad01")
        wo8 = big.tile([64, 4, D], fp8, tag="wo8", name="wo8")
        ones64 = consts.tile([65, 64], f32r, tag="ones64")
        nc.gpsimd.memset(ones64.bitcast(f32)[64:65, :], 1.0)
        if not _BV_ZERO:
            bv_bc = consts.tile([128, 256], f32, tag="bv")

        # per-chunk batched views: row s = c*512 + a*128 + p
        x_r = x_d.rearrange("(c a p) d -> c p a d", a=4, p=128)
        resid_r = resid_d.rearrange("(c a p) d -> c p a d", a=4, p=128)
        y_r = y_d.rearrange("(c a p) d -> c p a d", a=4, p=128)
        wqkv_sb = big.tile([128, 4, 768], fp8, tag="wqkv8", name="wqkv8")

        # persistent big tensors.  xnT8[p, g, s] = xn[s, 128g + p] in fp8:
        # each g-slot is one contiguous K=128 contraction block for the
        # plain-fp8 projection matmuls, written by one [128,128] PE
        # transpose per slot and evacuated at full 128-partition width.
        xnT8 = big.tile([128, 4, S], fp8, tag="xnT8", name="xnT8")
        qT = [big.tile([128, S], f32r, tag=f"qT{hp}", name=f"qT{hp}") for hp in range(2)]
        kT = [big.tile([128, S], f32r, tag=f"kT{hp}", name=f"kT{hp}") for hp in range(2)]
        v_sb = [big.tile([128, 4, 65], f32r, tag=f"v{st}", name=f"vsb{st}") for st in range(16)]
        # attention outputs, head h on partitions 0-63, slot h: outSt holds
        # raw numerators (2-chunk ring, consumed by the normalize drip one
        # block later), outT8 the normalized fp8 (2-chunk ring) feeding the
        # DoubleRow output projection.
        outSt = big.tile([64, 4, 2, 512], f32, tag="outSt", name="outSt")
        outT8 = big.tile([64, 4, 2, 512], fp8, tag="outT8", name="outT8")

        # ==== interleaved main loop: chunk blk of LN/proj, then the two
        # head-pairs' attention m=blk blocks (which need only chunks <= blk).
        # This keeps ScalarE (exp-bound) fed while PE does projections. ====
        from collections import deque

        pending_work = deque()
        x4_tiles = {}
        x0_tiles = []
        r4_tiles = {}
        dre_tiles = {}
        xn_map = {}

        def phaseA_ln(chv, ss):
            x_t = x0_tiles[ss] if chv == 0 else x4_tiles[chv][:, ss, :]
            stats = st_pool.tile([128, 6], f32, tag="stats", name="stats")
            nc.vector.bn_stats(out=stats, in_=x_t)
            mv = st_pool.tile([128, 2], f32, tag="mv", name="mv")
            nc.vector.bn_aggr(out=mv, in_=stats)
            lnv = st_pool.tile([128, 1], f32, tag="lnv", name="lnv")
            nc.scalar.activation(
                out=lnv, in_=mv[:, 1:2], func=AF.Ln, bias=eps_col, scale=1.0
            )
            rstd = st_pool.tile([128, 1], f32, tag="rstd", name="rstd")
            nc.scalar.activation(
                out=rstd, in_=lnv, func=AF.Exp, bias=0.0, scale=-0.5
            )
            xn_t = xn_pool.tile([128, D], bf16, tag="xn", name="xnt")
            # on Pool (gpsimd): all-SBUF op, and Pool is otherwise idle
            nc.gpsimd.tensor_scalar(
                out=xn_t,
                in0=x_t,
                scalar1=mv[:, 0:1],
                scalar2=rstd,
                op0=mybir.AluOpType.subtract,
                op1=mybir.AluOpType.mult,
            )
            xn_map[(chv, ss)] = xn_t

        def phaseA_tr(chv, ss):
            # transpose [128, 64] blocks: xn [s,d] -> xnT8 [64e+p, g, s].
            # Transposes run in bf16 (hw rejects fp8 transpose without
            # stride-2 packing); the evacuation copy casts to fp8, which
            # is what the DoubleRow projections consume.
            stv = chv * 4 + ss
            xn_t = xn_map.pop((chv, ss))
            ps_t = ps.tile([128, 4, 128], bf16, tag="pa", name="pst")
            # one [128,128] transpose per g-slot: out partition p = 64e+p64
            # lands d = 128g + 64e + p64, exactly the DoubleRow packing
            for g in range(4):
                nc.tensor.transpose(
                    out=ps_t[:, g, :],
                    in_=xn_t[:, g * 128 : (g + 1) * 128],
                    identity=ident,
                )
            if ss % 2 == 0:
                nc.scalar.copy(
                    out=xnT8[:, :, stv * 128 : (stv + 1) * 128], in_=ps_t
                )
            else:
                nc.vector.tensor_copy(
                    out=xnT8[:, :, stv * 128 : (stv + 1) * 128], in_=ps_t
                )

        DRPAIRS = [(0, 0), (0, 1), (1, 0), (1, 1)]

        def phaseA_qk(chv, jt):
            # q/k projection j-tile (q-pair0, q-pair1, k-pair0, k-pair1).
            # Plain fp8 matmuls: each g-slot covers contiguous d range
            # [128g, 128g+128) across the full 128 partitions (hw rejects
            # DoubleRow chains that mix base partitions 0/64).
            dst = qT[jt] if jt < 2 else kT[jt - 2]
            ps_qk = ps.tile([128, 512], f32, tag="pa", name="psqk")
            for g in range(4):
                nc.tensor.matmul(
                    out=ps_qk,
                    lhsT=wqkv_sb[:, g, jt * 128 : (jt + 1) * 128],
                    rhs=xnT8[:, g, chv * 512 : (chv + 1) * 512],
                    start=(g == 0),
                    stop=(g == 3),
                )
            nc.vector.tensor_scalar_add(
                out=dst[:, chv * 512 : (chv + 1) * 512],
                in0=ps_qk,
                scalar1=bcol[:, jt : jt + 1],
            )

        def phaseA_v(chv, ss):
            # v projection: [s, e] orientation with ones column + padding
            stv = chv * 4 + ss
            ps_v = ps.tile([128, 256], f32, tag="pa", name="psv")
            for g in range(4):
                nc.tensor.matmul(
                    out=ps_v,
                    lhsT=xnT8[:, g, stv * 128 : (stv + 1) * 128],
                    rhs=wqkv_sb[:, g, 512:768],
                    start=(g == 0),
                    stop=(g == 3),
                )
            vt = v_sb[stv]
            nc.gpsimd.memset(vt.bitcast(f32)[:, :, 64:65], 1.0)
            if _BV_ZERO:
                # (psum + 0) * pad in one op; the ones column is scaled
                # by a separate tiny op
                nc.vector.tensor_scalar_mul(
                    out=vt[:, :, 0:64],
                    in0=ps_v.rearrange("p (h e) -> p h e", h=4),
                    scalar1=pad01[:, stv : stv + 1],
                )
                nc.vector.tensor_scalar_mul(
                    out=vt[:, :, 64:65],
                    in0=vt[:, :, 64:65],
                    scalar1=pad01[:, stv : stv + 1],
                )
            else:
                nc.vector.tensor_tensor(
                    out=vt[:, :, 0:64],
                    in0=ps_v.rearrange("p (h e) -> p h e", h=4),
                    in1=bv_bc.rearrange("p (h e) -> p h e", h=4),
                    op=mybir.AluOpType.add,
                )
                nc.vector.tensor_scalar_mul(
                    out=vt[:, :, :],
                    in0=vt[:, :, :],
                    scalar1=pad01[:, stv : stv + 1],
                )

        def norm_row(h, mm):
            # rb = K=1 ones-matmul broadcast of the reciprocal row (psum
            # partition 64 -> partitions 0-63), then one DVE multiply
            # raw numerators -> normalized fp8 (DoubleRow outproj layout)
            hq, abq = divmod(h, 2)
            rb = ps.tile([64, 512], f32, tag="pa", name="rb")
            nc.tensor.matmul(
                out=rb,
                lhsT=ones64[64:65, :],
                rhs=dre_tiles[(hq, mm)][64:65, abq, :],
                start=True,
                stop=True,
            )
            nc.vector.tensor_tensor(
                out=outT8[:, h, mm % 2, :],
                in0=outSt[:, h, mm % 2, :],
                in1=rb,
                op=mybir.AluOpType.mult,
            )

        def load_resid(mm):
            r4 = r_pool.tile([128, 4, D], f32, tag="r4", name="r4")
            nc.sync.dma_start(out=r4, in_=resid_r[mm])
            r4_tiles[mm] = r4

        def outproj_tile(stv):
            mm, ssl = divmod(stv, 4)
            r4 = r4_tiles[mm]
            ps_y = ps.tile([128, 512], f32, tag="s", name="psy")
            for gp in range(2):
                nc.tensor.matmul(
                    out=ps_y,
                    lhsT=outT8[
                        :, 2 * gp : 2 * gp + 2, mm % 2,
                        ssl * 128 : (ssl + 1) * 128,
                    ],
                    rhs=wo8[:, 2 * gp : 2 * gp + 2, :],
                    start=(gp == 0),
                    stop=(gp == 1),
                    perf_mode=DR,
                )
            # y written in place over the residual slice; one DMA per
            # chunk (tail chunk: per-tile DMAs for latency)
            nc.vector.tensor_tensor(
                out=r4[:, ssl, :], in0=ps_y, in1=r4[:, ssl, :],
                op=mybir.AluOpType.add,
            )
            if mm == 3:
                nc.sync.dma_start(
                    out=y_d[stv * 128 : (stv + 1) * 128, :],
                    in_=r4[:, ssl, :],
                )
            elif ssl == 3:
                nc.sync.dma_start(out=y_r[mm], in_=r4)

        for blk in range(4):
            chv = blk
            # ---- phase A: chunk 0 runs inline before any attention; later
            # chunks are queued as pieces and popped during the PREVIOUS
            # block's attention j-loop so PE interleaves projection work
            # into the exp-gated attention chain ----
            if chv == 0:
                # chunk 0: four separate x DMAs so ss=0's LN starts as soon
                # as its 128 rows land; consts/weights queue behind them
                for ss in range(4):
                    x_t = xa_pool.tile([128, D], f32, tag="x", name="xt")
                    nc.sync.dma_start(
                        out=x_t, in_=x_d[ss * 128 : (ss + 1) * 128, :]
                    )
                    x0_tiles.append(x_t)
                make_identity(nc, ident)
                nc.sync.dma_start(out=bcol, in_=bcol_d[:, :])
                nc.sync.dma_start(out=pad01, in_=pad_d[:, :])
                if not _BV_ZERO:
                    nc.sync.dma_start(
                        out=bv_bc, in_=bv_d[None, :].to_broadcast([128, 256])
                    )
                nc.sync.dma_start(out=wqkv_sb, in_=wqkv_d[:, :, :])
                for ss in range(4):
                    phaseA_ln(0, ss)
                    phaseA_tr(0, ss)
                for jt in range(4):
                    phaseA_qk(0, jt)
                for ss in range(4):
                    phaseA_v(0, ss)
                # late-loaded weights (needed at outproj)
                nc.sync.dma_start(out=wo8, in_=wo_d[:, :, :])
            else:
                # safety: everything chunk-blk must be emitted by now
                while pending_work and pending_work[0][2] <= blk:
                    fn, args, _ = pending_work.popleft()
                    fn(*args)
            if chv < 3:
                # prefetch next chunk's x as one batched DMA, then queue its
                # phase A pieces (deadline: before block chv+1's attention)
                nxt = xa_pool.tile([128, 4, D], f32, tag="x4", bufs=2, name="x4")
                nc.sync.dma_start(out=nxt, in_=x_r[chv + 1])
                x4_tiles[chv + 1] = nxt
                d = chv + 1
                for ss in range(4):
                    pending_work.append((phaseA_ln, (d, ss), d))
                    pending_work.append((phaseA_tr, (d, ss), d))
                for jt in range(4):
                    pending_work.append((phaseA_qk, (d, jt), d))
                for ss in range(4):
                    pending_work.append((phaseA_v, (d, ss), d))

            # ---- attention m = blk for both head-pairs ----
            m = blk
            for hp in range(2):
                po = {}
                for ab in range(2):
                    po[ab] = ps.tile(
                        [65, 512], f32, tag=f"po{ab}", bufs=1, name=f"po{ab}"
                    )
                njt = 4 * m + 4  # key tiles 0 .. 4m+3
                for j in range(njt):
                    diag_o = 128 * (j - 4 * m) if j >= 4 * m else None
                    o = diag_o if diag_o is not None else 0
                    # both heads' scores in one 2-bank tile -> one fused exp;
                    # diagonal tiles restrict to query columns >= o (the
                    # sub-diagonal columns are never read downstream)
                    ps_s = ps.tile([128, 2, 512], f32, tag="s", name="pss")
                    for ab in range(2):
                        base = ab * 64
                        nc.tensor.matmul(
                            out=ps_s[:, ab, o:512],
                            lhsT=kT[hp][base : base + 64, j * 128 : (j + 1) * 128],
                            rhs=qT[hp][
                                base : base + 64, m * 512 + o : (m + 1) * 512
                            ],
                            start=True,
                            stop=True,
                        )
                    p_t = p_pool.tile([128, 2, 512], f32r, tag="p", name="pt")
                    nc.scalar.activation(
                        out=p_t[:, :, o:512],
                        in_=ps_s[:, :, o:512],
                        func=AF.Exp,
                        bias=0.0,
                        scale=0.125,
                    )
                    if diag_o is not None:
                        # zero strictly-below-diagonal: keep col-part>=0
                        nc.gpsimd.affine_select(
                            out=p_t[:, :, o : o + 128],
                            in_=p_t[:, :, o : o + 128],
                            compare_op=mybir.AluOpType.is_ge,
                            fill=0.0,
                            base=0,
                            pattern=[[0, 2], [1, 128]],
                            channel_multiplier=-1,
                        )
                    for ab in range(2):
                        h = hp * 2 + ab
                        nc.tensor.matmul(
                            out=po[ab][:, o:512],
                            lhsT=v_sb[j][:, h, 0:65],
                            rhs=p_t[:, ab, o:512],
                            start=(j == 0),
                            stop=(j == njt - 1),
                        )
                    # skip pops on the last two j's: keeps the DVE queue
                    # short so the po evacuation (which gates the next
                    # block's first PV via po bufs=1) isn't queued behind
                    # dripped elementwise work
                    if j < njt - 2:
                        for _ in range(2):
                            if pending_work:
                                fn, args, _d = pending_work.popleft()
                                fn(*args)
                # reciprocal the denominator rows straight out of psum
                # partition 64 into dre (broadcast later by a K=1 ones
                # matmul in the normalize drip), then evacuate the raw
                # numerators to outSt (ring slot m%2); recips first so the
                # m=3 tail's normalize chain starts as early as possible
                dre = dt_pool.tile([65, 2, 512], f32r, tag="dre", name="dre")
                with nc.allow_low_precision(reason="f32r recip row for PE broadcast"):
                    for ab in range(2):
                        nc.vector.reciprocal(
                            out=dre[64:65, ab, :],
                            in_=po[ab][64:65, :],
                        )
                dre_tiles[(hp, m)] = dre
                for ab in range(2):
                    if blk == 3:
                        # tail: ACT is idle here, and this shortens the
                        # DVE chain gating the last outproj
                        nc.scalar.copy(
                            out=outSt[:, hp * 2 + ab, m % 2, :],
                            in_=po[ab][0:64, :],
                        )
                    else:
                        nc.vector.tensor_copy(
                            out=outSt[:, hp * 2 + ab, m % 2, :],
                            in_=po[ab][0:64, :],
                        )

                if blk == 3:
                    # m=3 normalize immediately (short tail); chunk-3
                    # outproj after both head-pairs are in
                    for ab in range(2):
                        norm_row(hp * 2 + ab, 3)
                    if hp == 1:
                        while pending_work:
                            fn, args, _d = pending_work.popleft()
                            fn(*args)
                        for ss in range(4):
                            outproj_tile(12 + ss)

            # ---- queue chunk-m normalize + output projection; popped
            # during block m+1's attention j-loop ----
            if blk < 3:
                d = blk + 2
                for h_ in range(4):
                    pending_work.append((norm_row, (h_, blk), d))
                pending_work.append((load_resid, (blk,), d))
                for s_ in range(4):
                    pending_work.append((outproj_tile, (blk * 4 + s_,), d))
                if blk == 2:
                    pending_work.append((load_resid, (3,), 4))

    return nc


def _get_program(bv_zero=False):
    if bv_zero not in _PROGRAM:
        _PROGRAM[bv_zero] = _build_program(bv_zero)
    return _PROGRAM[bv_zero]


def _make_in_maps(inputs):
    x = np.ascontiguousarray(np.asarray(inputs["x"], dtype=np.float32))
    lengths = np.asarray(inputs["key_value_sequence_lengths"]).astype(np.int64)
    Wq = np.asarray(inputs["Wq"], dtype=np.float32)
    bq = np.asarray(inputs["bq"], dtype=np.float32)
    Wkv = np.asarray(inputs["Wkv"], dtype=np.float32)
    bkv = np.asarray(inputs["bkv"], dtype=np.float32)
    Wo = np.asarray(inputs["Wo"], dtype=np.float32)
    bo = np.asarray(inputs["bo"], dtype=np.float32)
    gamma = np.asarray(inputs["gamma"], dtype=np.float32)
    beta = np.asarray(inputs["beta"], dtype=np.float32)

    H = 8
    Wk = Wkv[:, : H * DQ]
    Wv = Wkv[:, H * DQ :]
    bk = bkv[: H * DQ]
    bv_full = bkv[H * DQ :]

    in_maps = []
    for c in range(N_CORES):
        n = c // 2
        h0 = 4 * (c % 2)
        hsel = slice(h0 * DQ, (h0 + 4) * DQ)  # 256 contiguous columns

        wq_s = Wq[:, hsel]
        wk_s = Wk[:, hsel]
        wv_s = Wv[:, hsel]
        import ml_dtypes

        wqkv = np.concatenate(
            [gamma[:, None] * wq_s, gamma[:, None] * wk_s, gamma[:, None] * wv_s],
            axis=1,
        )
        # [128, 4, 768]: wqkv8[p, g, c] = wqkv[128g + p, c] (g-slot = one
        # contiguous K=128 contraction block)
        wqkv = np.ascontiguousarray(
            wqkv.reshape(4, 128, 768).transpose(1, 0, 2)
        ).astype(ml_dtypes.float8_e4m3)
        bq_eff = beta @ wq_s + bq[hsel]
        bk_eff = beta @ wk_s + bk[hsel]
        bv_eff = beta @ wv_s + bv_full[hsel]
        bcol = np.concatenate([bq_eff, bk_eff]).reshape(4, 128).T.copy()
        # [64, 4, 512]: wo8[p, h, d] = Wo[64h+p, d] (DoubleRow head pairs)
        wo_s = np.ascontiguousarray(
            Wo[hsel, :].reshape(4, 64, D).transpose(1, 0, 2)
        ).astype(ml_dtypes.float8_e4m3)

        ln = int(lengths[n])
        b_idx = np.arange(128)[:, None]
        j_idx = np.arange(16)[None, :]
        pad01 = ((128 * j_idx + b_idx) < ln).astype(np.float32)

        resid = x[n] if c % 2 == 0 else np.ascontiguousarray(
            np.broadcast_to(bo, (S, D)).astype(np.float32)
        )

        in_maps.append(
            {
                "x": x[n],
                "resid": resid,
                "wqkv": wqkv,
                "bcol": np.ascontiguousarray(bcol, dtype=np.float32),
                "bv": bv_eff.astype(np.float32),
                "wo": wo_s,
                "pad01": np.ascontiguousarray(pad01, dtype=np.float32),
            }
        )
    return in_maps


def kernel_run(inputs, trace=False):
    from concourse.bass_utils import run_bass_kernel_spmd

    in_maps = _make_in_maps(inputs)
    bv_zero = all(
        not np.any(np.asarray(m["bv"], dtype=np.float32)) for m in in_maps
    )
    nc = _get_program(bv_zero)
    res = run_bass_kernel_spmd(nc, in_maps, list(range(N_CORES)), trace=trace)
    parts = [res.results[c]["y"] for c in range(N_CORES)]
    out = np.stack(
        [parts[2 * n] + parts[2 * n + 1] for n in range(4)], axis=0
    ).astype(np.float32)
    return out, res


def kernel(**inputs) -> np.ndarray:
    out, _ = kernel_run(inputs)
    return out



# revision 62
# speedup vs baseline: 1.2152x; 1.0449x over previous
"""Trainium2 Bass/Tile kernel for pre-LN causal multi-head self-attention.

Problem shapes (hardcoded): x (4, 2048, 512), 8 heads, dq=dv=64, fp32.

Sharding over 8 NeuronCores: core c handles batch n = c//2 and the 4 heads
h in [4*(c%2), 4*(c%2)+4).  Every core runs the SAME program (SPMD); all
per-core differences are carried by input values:
  - x:     the core's batch (2048, 512)
  - resid: even cores: x[n] (residual); odd cores: broadcast bo rows.
           Each term of (residual + bo) is added exactly once per pair.
  - wqkv:  (128, 4, 768) fp8e4 gamma-folded [Wq | Wk | Wv], g-slot packed:
           wqkv[p, g, c] = W[128g + p, c]
  - bcol:  (128, 4) q/k projection bias columns (beta @ W + b)
  - bv:    (256,) v-projection bias (zero-specialized when all-zero)
  - wo:    (64, 4, 512) fp8e4 Wo, wo[p, h, d] = Wo[64h + p, d]
  - pad01: (128, 16) key-padding mask column per key tile
Host combines: out[n] = y_part[2n] + y_part[2n+1].

Dataflow:
  LN:      bn_stats/bn_aggr on DVE; rstd = Exp(-0.5*Ln(var+eps)) on ACT
           (one table set); xn = (x-mu)*rstd in bf16 on GPSIMD.
  xnT8:    one [128,128] PE transpose per 128-d block -> psum bf16,
           evacuated (ACT/DVE alternating) with a cast to fp8:
           xnT8[p, g, s] = xn[s, 128g+p].
  q/k/v:   plain fp8 matmuls, one per g-slot (K=128 each; hw rejects
           DoubleRow chains mixing base partitions 0/64).  qT/kT f32r
           [128, S] (2 heads per tile on partition halves); v [s, 4, 65]
           f32r tiles with a ones column, padded key rows zeroed.
  scores:  sT[key, query] = kT^T @ qT (f32r, K=64), both heads into one
           2-bank psum tile -> ONE fused exp [128, 2, 512-o] per key tile
           (scale=1/8 folded in); diagonal tiles restrict to columns >= o
           and zero the sub-diagonal via GPSIMD affine_select.
  PV:      out'[65, 512] += v'^T @ p accumulated in psum; row 64 collects
           softmax denominators via the ones column.
  norm:    DVE reciprocal reads the denominator row straight from psum
           partition 64 into dre (f32r); a K=1 ones-matmul broadcasts it
           to partitions 0-63; one DVE multiply writes normalized fp8
           outT8 [64, 4(head), 2(ring), 512] from raw outSt.
  outproj: two fp8 DoubleRow matmuls (head pairs (0,1),(2,3), base 0) +
           in-place residual add on DVE; y DMA'd per chunk (per tile for
           the tail chunk).

Schedule: phase A (LN/transpose/projections) of chunk m+1 and the
normalize+outproj of chunk m-1 are queued as work items and popped into
block m's attention j-loop (deadline-checked), so PE interleaves them
into the exp-paced attention chain; x/resid/y move as per-chunk batched
DMAs (x chunk 0 split per-tile for a fast LN start).  One shared PSUM
pool budgets exactly 8 banks: pa(2) + s(2x2) + po0(1) + po1(1), with
rb->pa and psy->s tag reuse.  fp8 noise ~6-9e-3 rel vs the 2e-2 gate.
"""

import numpy as np

S = 2048
D = 512
DQ = 64
H_PER_CORE = 4
N_CORES = 8
LN_EPS = 1e-5
NEG = -30.0

_PROGRAM = {}


def _install_tile_patch():
    """Workarounds for walrus/concourse skew in this container:

    1. This walrus build rejects instructions carrying more than one
       semaphore-wait command ("Too many sync wait commands"), but Tile's
       rust wait-assigner freely emits 2-3 waits per instruction.  After
       wait assignment, split excess waits onto EventSemaphore carrier
       instructions inserted just before the owner on the same engine.
    2. Tile's tail drain carries one wait per outstanding proc; split into
       one drain per proc.
    3. Tile's tail emits a gpsimd sem_clear (Pool ISA opcode 176) that this
       walrus rejects ("ISA wrong length").  The NRT preamble's sema_reset
       zeroes user semaphores at every execution, so the tail clear is
       redundant and skipped.
    """
    from bass_rust import SyncInfo as _SyncInfo

    from concourse import mybir, tile
    from concourse.vector_clock import ScopedClock, VectorClock

    if getattr(tile.TileContext, "_mha_patch", False):
        return

    MAXW = 1

    def _split_excess_waits(ordered, nc):
        for bb_name, insts in list(ordered.items()):
            out = []
            changed = False
            for inst in insts:
                si = inst.sync_info
                if si is None:
                    out.append(inst)
                    continue
                waits = list(si.on_wait)
                if len(waits) > MAXW:
                    changed = True
                    excess = waits[:-MAXW]
                    for k in range(0, len(excess), MAXW):
                        carrier = mybir.InstEventSemaphore(
                            name=f"wsplit-{nc.next_id()}"
                        )
                        carrier.engine = inst.engine
                        carrier.bass_scheduled_proc = inst.bass_scheduled_proc
                        carrier.bass_scheduled_scope = inst.bass_scheduled_scope
                        carrier.bass_scheduled_tick = inst.bass_scheduled_tick
                        carrier.sync_info = _SyncInfo(
                            on_wait=excess[k : k + MAXW], on_update=[]
                        )
                        out.append(carrier)
                    si.on_wait = waits[-MAXW:]
                    inst.sync_info = si
                out.append(inst)
            if changed:
                ordered[bb_name] = out

    _RustTileClockWait = tile.TileClockWait

    class _SplittingTileClockWait:
        def __init__(self, tc, ordered, **kw):
            self._inner = _RustTileClockWait(tc, ordered, **kw)
            self._ordered = ordered
            self._nc = tc.nc

        def __getattr__(self, k):
            return getattr(self._inner, k)

        def assign_waits(self, bb_name):
            self._inner.assign_waits(bb_name)
            _split_excess_waits(self._ordered, self._nc)

    tile.TileClockWait = _SplittingTileClockWait

    def _patched_drain_and_barrier(self, tick_clock, wait_clock):
        gvec = tick_clock.global_clock
        n = len(gvec)
        for i in range(n):
            if gvec[i] > 0:
                v = VectorClock([gvec[j] if j == i else 0 for j in range(n)])
                d = self.nc.sync.drain()
                wait_clock.add_sem_waits(d.ins, ScopedClock({None: v}))
        self.nc.all_engine_barrier()
        assert self.sems is not None
        popped = self.nc._tile_sem_poison_stack.pop()
        assert popped is self._sem_poison
        self.nc.all_engine_barrier()

    tile.TileContext._drain_and_barrier = _patched_drain_and_barrier

    # cayman has 208 KiB usable per partition; the stale 192 KiB constant
    # leaves 16 KiB on the table
    from concourse import tile_utils

    tile_utils.max_sbuf_usage = 208 * 1024
    tile.TileContext._mha_patch = True


def _build_program(bv_zero=False):
    _BV_ZERO = bv_zero
    from contextlib import ExitStack

    import concourse.bass as bass
    import concourse.tile as tile
    from concourse import mybir
    from concourse.masks import make_identity

    f32 = mybir.dt.float32
    f32r = mybir.dt.float32r
    bf16 = mybir.dt.bfloat16
    fp8 = mybir.dt.float8e4
    DR = mybir.MatmulPerfMode.DoubleRow
    AF = mybir.ActivationFunctionType

    nc = bass.Bass()

    x_d = nc.dram_tensor("x", [S, D], f32, kind="ExternalInput")
    resid_d = nc.dram_tensor("resid", [S, D], f32, kind="ExternalInput")
    wqkv_d = nc.dram_tensor(
        "wqkv", [128, 4, 768], mybir.dt.float8e4, kind="ExternalInput"
    )
    bcol_d = nc.dram_tensor("bcol", [128, 4], f32, kind="ExternalInput")
    bv_d = nc.dram_tensor("bv", [256], f32, kind="ExternalInput")
    wo_d = nc.dram_tensor("wo", [64, 4, D], mybir.dt.float8e4, kind="ExternalInput")
    pad_d = nc.dram_tensor("pad01", [128, 16], f32, kind="ExternalInput")
    y_d = nc.dram_tensor("y", [S, D], f32, kind="ExternalOutput")

    _install_tile_patch()

    with tile.TileContext(nc) as tc, ExitStack() as ctx:
        consts = ctx.enter_context(tc.tile_pool(name="consts", bufs=1))
        big = ctx.enter_context(tc.tile_pool(name="big", bufs=1))
        # one shared PSUM pool; tag budget adds up to exactly 8 banks so all
        # phases can be in flight at once:
        #   pa(2) + sA(2) + sB(2) + poA(1) + poB(1) = 8
        # (rb reuses sA slots, psum_y reuses sB slots later in the kernel)
        ps = ctx.enter_context(tc.tile_pool(name="ps", bufs=2, space="PSUM"))
        xa_pool = ctx.enter_context(tc.tile_pool(name="xa", bufs=4))
        xn_pool = ctx.enter_context(tc.tile_pool(name="xn", bufs=8))
        st_pool = ctx.enter_context(tc.tile_pool(name="st", bufs=8))
        p_pool = ctx.enter_context(tc.tile_pool(name="pp", bufs=5))
        dt_pool = ctx.enter_context(tc.tile_pool(name="dt", bufs=5))
        r_pool = ctx.enter_context(tc.tile_pool(name="rr", bufs=2))

        # ---- constants and weights (x-chunk0 DMAs are issued FIRST in the
        # blk-0 body; all const DMAs are emitted after them so the LN start
        # isn't queued behind them on the single HWDGE ring) ----
        ident = consts.tile([128, 128], bf16, tag="ident")
        eps_col = consts.tile([128, 1], f32, tag="eps")
        nc.vector.memset(eps_col, LN_EPS)
        bcol = consts.tile([128, 4], f32, tag="bcol")
        pad01 = consts.tile([128, 16], f32, tag="pad01")
        wo8 = big.tile([64, 4, D], fp8, tag="wo8", name="wo8")
        ones64 = consts.tile([65, 64], f32r, tag="ones64")
        nc.gpsimd.memset(ones64.bitcast(f32)[64:65, :], 1.0)
        if not _BV_ZERO:
            bv_bc = consts.tile([128, 256], f32, tag="bv")

        # per-chunk batched views: row s = c*512 + a*128 + p
        x_r = x_d.rearrange("(c a p) d -> c p a d", a=4, p=128)
        resid_r = resid_d.rearrange("(c a p) d -> c p a d", a=4, p=128)
        y_r = y_d.rearrange("(c a p) d -> c p a d", a=4, p=128)
        wqkv_sb = big.tile([128, 4, 768], fp8, tag="wqkv8", name="wqkv8")

        # persistent big tensors.  xnT8[p, g, s] = xn[s, 128g + p] in fp8:
        # each g-slot is one contiguous K=128 contraction block for the
        # plain-fp8 projection matmuls, written by one [128,128] PE
        # transpose per slot and evacuated at full 128-partition width.
        xnT8 = big.tile([128, 4, S], fp8, tag="xnT8", name="xnT8")
        qT = [big.tile([128, S], f32r, tag=f"qT{hp}", name=f"qT{hp}") for hp in range(2)]
        kT = [big.tile([128, S], f32r, tag=f"kT{hp}", name=f"kT{hp}") for hp in range(2)]
        v_sb = [big.tile([128, 4, 65], f32r, tag=f"v{st}", name=f"vsb{st}") for st in range(16)]
        # attention outputs, head h on partitions 0-63, slot h: outSt holds
        # raw numerators (2-chunk ring, consumed by the normalize drip one
        # block later), outT8 the normalized fp8 (2-chunk ring) feeding the
        # DoubleRow output projection.
        outSt = big.tile([64, 4, 2, 512], f32, tag="outSt", name="outSt")
        outT8 = big.tile([64, 4, 2, 512], fp8, tag="outT8", name="outT8")

        # ==== interleaved main loop: chunk blk of LN/proj, then the two
        # head-pairs' attention m=blk blocks (which need only chunks <= blk).
        # This keeps ScalarE (exp-bound) fed while PE does projections. ====
        from collections import deque

        pending_work = deque()
        x4_tiles = {}
        x0_tiles = []
        r4_tiles = {}
        dre_tiles = {}
        xn_map = {}

        def phaseA_ln(chv, ss):
            x_t = x0_tiles[ss] if chv == 0 else x4_tiles[chv][:, ss, :]
            stats = st_pool.tile([128, 6], f32, tag="stats", name="stats")
            nc.vector.bn_stats(out=stats, in_=x_t)
            mv = st_pool.tile([128, 2], f32, tag="mv", name="mv")
            nc.vector.bn_aggr(out=mv, in_=stats)
            lnv = st_pool.tile([128, 1], f32, tag="lnv", name="lnv")
            nc.scalar.activation(
                out=lnv, in_=mv[:, 1:2], func=AF.Ln, bias=eps_col, scale=1.0
            )
            rstd = st_pool.tile([128, 1], f32, tag="rstd", name="rstd")
            nc.scalar.activation(
                out=rstd, in_=lnv, func=AF.Exp, bias=0.0, scale=-0.5
            )
            xn_t = xn_pool.tile([128, D], bf16, tag="xn", name="xnt")
            # on Pool (gpsimd): all-SBUF op, and Pool is otherwise idle
            nc.gpsimd.tensor_scalar(
                out=xn_t,
                in0=x_t,
                scalar1=mv[:, 0:1],
                scalar2=rstd,
                op0=mybir.AluOpType.subtract,
                op1=mybir.AluOpType.mult,
            )
            xn_map[(chv, ss)] = xn_t

        def phaseA_tr(chv, ss):
            # transpose [128, 64] blocks: xn [s,d] -> xnT8 [64e+p, g, s].
            # Transposes run in bf16 (hw rejects fp8 transpose without
            # stride-2 packing); the evacuation copy casts to fp8, which
            # is what the DoubleRow projections consume.
            stv = chv * 4 + ss
            xn_t = xn_map.pop((chv, ss))
            ps_t = ps.tile([128, 4, 128], bf16, tag="pa", name="pst")
            # one [128,128] transpose per g-slot: out partition p = 64e+p64
            # lands d = 128g + 64e + p64, exactly the DoubleRow packing
            for g in range(4):
                nc.tensor.transpose(
                    out=ps_t[:, g, :],
                    in_=xn_t[:, g * 128 : (g + 1) * 128],
                    identity=ident,
                )
            if ss % 2 == 0:
                nc.scalar.copy(
                    out=xnT8[:, :, stv * 128 : (stv + 1) * 128], in_=ps_t
                )
            else:
                nc.vector.tensor_copy(
                    out=xnT8[:, :, stv * 128 : (stv + 1) * 128], in_=ps_t
                )

        DRPAIRS = [(0, 0), (0, 1), (1, 0), (1, 1)]

        def phaseA_qk(chv, jt):
            # q/k projection j-tile (q-pair0, q-pair1, k-pair0, k-pair1).
            # Plain fp8 matmuls: each g-slot covers contiguous d range
            # [128g, 128g+128) across the full 128 partitions (hw rejects
            # DoubleRow chains that mix base partitions 0/64).
            dst = qT[jt] if jt < 2 else kT[jt - 2]
            ps_qk = ps.tile([128, 512], f32, tag="pa", name="psqk")
            for g in range(4):
                nc.tensor.matmul(
                    out=ps_qk,
                    lhsT=wqkv_sb[:, g, jt * 128 : (jt + 1) * 128],
                    rhs=xnT8[:, g, chv * 512 : (chv + 1) * 512],
                    start=(g == 0),
                    stop=(g == 3),
                )
            nc.vector.tensor_scalar_add(
                out=dst[:, chv * 512 : (chv + 1) * 512],
                in0=ps_qk,
                scalar1=bcol[:, jt : jt + 1],
            )

        def phaseA_v(chv, ss):
            # v projection: [s, e] orientation with ones column + padding
            stv = chv * 4 + ss
            ps_v = ps.tile([128, 256], f32, tag="pa", name="psv")
            for g in range(4):
                nc.tensor.matmul(
                    out=ps_v,
                    lhsT=xnT8[:, g, stv * 128 : (stv + 1) * 128],
                    rhs=wqkv_sb[:, g, 512:768],
                    start=(g == 0),
                    stop=(g == 3),
                )
            vt = v_sb[stv]
            nc.gpsimd.memset(vt.bitcast(f32)[:, :, 64:65], 1.0)
            if _BV_ZERO:
                # (psum + 0) * pad in one op; the ones column is scaled
                # by a separate tiny op
                nc.vector.tensor_scalar_mul(
                    out=vt[:, :, 0:64],
                    in0=ps_v.rearrange("p (h e) -> p h e", h=4),
                    scalar1=pad01[:, stv : stv + 1],
                )
                nc.vector.tensor_scalar_mul(
                    out=vt[:, :, 64:65],
                    in0=vt[:, :, 64:65],
                    scalar1=pad01[:, stv : stv + 1],
                )
            else:
                nc.vector.tensor_tensor(
                    out=vt[:, :, 0:64],
                    in0=ps_v.rearrange("p (h e) -> p h e", h=4),
                    in1=bv_bc.rearrange("p (h e) -> p h e", h=4),
                    op=mybir.AluOpType.add,
                )
                nc.vector.tensor_scalar_mul(
                    out=vt[:, :, :],
                    in0=vt[:, :, :],
                    scalar1=pad01[:, stv : stv + 1],
                )

        def norm_row(h, mm):
            # rb = K=1 ones-matmul broadcast of the reciprocal row (psum
            # partition 64 -> partitions 0-63), then one DVE multiply
            # raw numerators -> normalized fp8 (DoubleRow outproj layout)
            hq, abq = divmod(h, 2)
            rb = ps.tile([64, 512], f32, tag="pa", name="rb")
            nc.tensor.matmul(
                out=rb,
                lhsT=ones64[64:65, :],
                rhs=dre_tiles[(hq, mm)][64:65, abq, :],
                start=True,
                stop=True,
            )
            nc.vector.tensor_tensor(
                out=outT8[:, h, mm % 2, :],
                in0=outSt[:, h, mm % 2, :],
                in1=rb,
                op=mybir.AluOpType.mult,
            )

        def load_resid(mm):
            r4 = r_pool.tile([128, 4, D], f32, tag="r4", name="r4")
            nc.sync.dma_start(out=r4, in_=resid_r[mm])
            r4_tiles[mm] = r4

        def outproj_tile(stv):
            mm, ssl = divmod(stv, 4)
            r4 = r4_tiles[mm]
            ps_y = ps.tile([128, 512], f32, tag="s", name="psy")
            for gp in range(2):
                nc.tensor.matmul(
                    out=ps_y,
                    lhsT=outT8[
                        :, 2 * gp : 2 * gp + 2, mm % 2,
                        ssl * 128 : (ssl + 1) * 128,
                    ],
                    rhs=wo8[:, 2 * gp : 2 * gp + 2, :],
                    start=(gp == 0),
                    stop=(gp == 1),
                    perf_mode=DR,
                )
            # y written in place over the residual slice; one DMA per
            # chunk (tail chunk: per-tile DMAs for latency)
            nc.vector.tensor_tensor(
                out=r4[:, ssl, :], in0=ps_y, in1=r4[:, ssl, :],
                op=mybir.AluOpType.add,
            )
            if mm == 3:
                nc.sync.dma_start(
                    out=y_d[stv * 128 : (stv + 1) * 128, :],
                    in_=r4[:, ssl, :],
                )
            elif ssl == 3:
                nc.sync.dma_start(out=y_r[mm], in_=r4)

        for blk in range(4):
            chv = blk
            # ---- phase A: chunk 0 runs inline before any attention; later
            # chunks are queued as pieces and popped during the PREVIOUS
            # block's attention j-loop so PE interleaves projection work
            # into the exp-gated attention chain ----
            if chv == 0:
                # chunk 0: four separate x DMAs so ss=0's LN starts as soon
                # as its 128 rows land; consts/weights queue behind them
                for ss in range(4):
                    x_t = xa_pool.tile([128, D], f32, tag="x", name="xt")
                    nc.sync.dma_start(
                        out=x_t, in_=x_d[ss * 128 : (ss + 1) * 128, :]
                    )
                    x0_tiles.append(x_t)
                make_identity(nc, ident)
                nc.sync.dma_start(out=wqkv_sb, in_=wqkv_d[:, :, :])
                nc.sync.dma_start(out=bcol, in_=bcol_d[:, :])
                nc.sync.dma_start(out=pad01, in_=pad_d[:, :])
                if not _BV_ZERO:
                    nc.sync.dma_start(
                        out=bv_bc, in_=bv_d[None, :].to_broadcast([128, 256])
                    )
                for ss in range(4):
                    phaseA_ln(0, ss)
                    phaseA_tr(0, ss)
                for jt in range(4):
                    phaseA_qk(0, jt)
                for ss in range(4):
                    phaseA_v(0, ss)
                # late-loaded weights (needed at outproj)
                nc.sync.dma_start(out=wo8, in_=wo_d[:, :, :])
            else:
                # safety: everything chunk-blk must be emitted by now
                while pending_work and pending_work[0][2] <= blk:
                    fn, args, _ = pending_work.popleft()
                    fn(*args)
            if chv < 3:
                # prefetch next chunk's x as one batched DMA, then queue its
                # phase A pieces (deadline: before block chv+1's attention)
                nxt = xa_pool.tile([128, 4, D], f32, tag="x4", bufs=2, name="x4")
                nc.sync.dma_start(out=nxt, in_=x_r[chv + 1])
                x4_tiles[chv + 1] = nxt
                d = chv + 1
                for ss in range(4):
                    pending_work.append((phaseA_ln, (d, ss), d))
                    pending_work.append((phaseA_tr, (d, ss), d))
                for jt in range(4):
                    pending_work.append((phaseA_qk, (d, jt), d))
                for ss in range(4):
                    pending_work.append((phaseA_v, (d, ss), d))

            # ---- attention m = blk for both head-pairs ----
            m = blk
            for hp in range(2):
                po = {}
                for ab in range(2):
                    po[ab] = ps.tile(
                        [65, 512], f32, tag=f"po{ab}", bufs=1, name=f"po{ab}"
                    )
                njt = 4 * m + 4  # key tiles 0 .. 4m+3
                for j in range(njt):
                    diag_o = 128 * (j - 4 * m) if j >= 4 * m else None
                    o = diag_o if diag_o is not None else 0
                    # both heads' scores in one 2-bank tile -> one fused exp;
                    # diagonal tiles restrict to query columns >= o (the
                    # sub-diagonal columns are never read downstream)
                    ps_s = ps.tile([128, 2, 512], f32, tag="s", name="pss")
                    for ab in range(2):
                        base = ab * 64
                        nc.tensor.matmul(
                            out=ps_s[:, ab, o:512],
                            lhsT=kT[hp][base : base + 64, j * 128 : (j + 1) * 128],
                            rhs=qT[hp][
                                base : base + 64, m * 512 + o : (m + 1) * 512
                            ],
                            start=True,
                            stop=True,
                        )
                    p_t = p_pool.tile([128, 2, 512], f32r, tag="p", name="pt")
                    nc.scalar.activation(
                        out=p_t[:, :, o:512],
                        in_=ps_s[:, :, o:512],
                        func=AF.Exp,
                        bias=0.0,
                        scale=0.125,
                    )
                    if diag_o is not None:
                        # zero strictly-below-diagonal: keep col-part>=0
                        nc.gpsimd.affine_select(
                            out=p_t[:, :, o : o + 128],
                            in_=p_t[:, :, o : o + 128],
                            compare_op=mybir.AluOpType.is_ge,
                            fill=0.0,
                            base=0,
                            pattern=[[0, 2], [1, 128]],
                            channel_multiplier=-1,
                        )
                    for ab in range(2):
                        h = hp * 2 + ab
                        nc.tensor.matmul(
                            out=po[ab][:, o:512],
                            lhsT=v_sb[j][:, h, 0:65],
                            rhs=p_t[:, ab, o:512],
                            start=(j == 0),
                            stop=(j == njt - 1),
                        )
                    # blk 0's short loop needs 2 pops/j to cover chunk 1's
                    # 16 phase-A pieces; later blocks pace at 1/j.  blk 3
                    # halves the rate again so the mm=2 drips also fill
                    # the second head-pair's exp-paced PE gaps.
                    npop = 2 if blk == 0 else (1 if blk < 3 else (j + hp) % 2)
                    for _ in range(npop):
                        if pending_work:
                            fn, args, _d = pending_work.popleft()
                            fn(*args)
                # reciprocal the denominator rows straight out of psum
                # partition 64 into dre (broadcast later by a K=1 ones
                # matmul in the normalize drip), then evacuate the raw
                # numerators to outSt (ring slot m%2); recips first so the
                # m=3 tail's normalize chain starts as early as possible
                dre = dt_pool.tile([65, 2, 512], f32r, tag="dre", name="dre")
                with nc.allow_low_precision(reason="f32r recip row for PE broadcast"):
                    for ab in range(2):
                        nc.vector.reciprocal(
                            out=dre[64:65, ab, :],
                            in_=po[ab][64:65, :],
                        )
                dre_tiles[(hp, m)] = dre
                for ab in range(2):
                    if blk == 3:
                        # tail: ACT is idle here, and this shortens the
                        # DVE chain gating the last outproj
                        nc.scalar.copy(
                            out=outSt[:, hp * 2 + ab, m % 2, :],
                            in_=po[ab][0:64, :],
                        )
                    else:
                        nc.vector.tensor_copy(
                            out=outSt[:, hp * 2 + ab, m % 2, :],
                            in_=po[ab][0:64, :],
                        )

                if blk == 3:
                    # m=3 normalize immediately (short tail); both rb
                    # broadcasts first so the DVE multiplies pipeline
                    rbs = {}
                    for ab in range(2):
                        rb = ps.tile([64, 512], f32, tag="pa", name="rb")
                        nc.tensor.matmul(
                            out=rb,
                            lhsT=ones64[64:65, :],
                            rhs=dre[64:65, ab, :],
                            start=True,
                            stop=True,
                        )
                        rbs[ab] = rb
                    for ab in range(2):
                        h = hp * 2 + ab
                        nc.vector.tensor_tensor(
                            out=outT8[:, h, 1, :],
                            in0=outSt[:, h, 1, :],
                            in1=rbs[ab],
                            op=mybir.AluOpType.mult,
                        )
                    if hp == 1:
                        while pending_work:
                            fn, args, _d = pending_work.popleft()
                            fn(*args)
                        for ss in range(4):
                            outproj_tile(12 + ss)

            # ---- queue chunk-m normalize + output projection; popped
            # during block m+1's attention j-loop ----
            if blk < 3:
                d = blk + 2
                for h_ in range(4):
                    pending_work.append((norm_row, (h_, blk), d))
                pending_work.append((load_resid, (blk,), d))
                for s_ in range(4):
                    pending_work.append((outproj_tile, (blk * 4 + s_,), d))
                if blk == 2:
                    pending_work.append((load_resid, (3,), 4))

    return nc


def _get_program(bv_zero=False):
    if bv_zero not in _PROGRAM:
        _PROGRAM[bv_zero] = _build_program(bv_zero)
    return _PROGRAM[bv_zero]


def _make_in_maps(inputs):
    x = np.ascontiguousarray(np.asarray(inputs["x"], dtype=np.float32))
    lengths = np.asarray(inputs["key_value_sequence_lengths"]).astype(np.int64)
    Wq = np.asarray(inputs["Wq"], dtype=np.float32)
    bq = np.asarray(inputs["bq"], dtype=np.float32)
    Wkv = np.asarray(inputs["Wkv"], dtype=np.float32)
    bkv = np.asarray(inputs["bkv"], dtype=np.float32)
    Wo = np.asarray(inputs["Wo"], dtype=np.float32)
    bo = np.asarray(inputs["bo"], dtype=np.float32)
    gamma = np.asarray(inputs["gamma"], dtype=np.float32)
    beta = np.asarray(inputs["beta"], dtype=np.float32)

    H = 8
    Wk = Wkv[:, : H * DQ]
    Wv = Wkv[:, H * DQ :]
    bk = bkv[: H * DQ]
    bv_full = bkv[H * DQ :]

    in_maps = []
    for c in range(N_CORES):
        n = c // 2
        h0 = 4 * (c % 2)
        hsel = slice(h0 * DQ, (h0 + 4) * DQ)  # 256 contiguous columns

        wq_s = Wq[:, hsel]
        wk_s = Wk[:, hsel]
        wv_s = Wv[:, hsel]
        import ml_dtypes

        wqkv = np.concatenate(
            [gamma[:, None] * wq_s, gamma[:, None] * wk_s, gamma[:, None] * wv_s],
            axis=1,
        )
        # [128, 4, 768]: wqkv8[p, g, c] = wqkv[128g + p, c] (g-slot = one
        # contiguous K=128 contraction block)
        wqkv = np.ascontiguousarray(
            wqkv.reshape(4, 128, 768).transpose(1, 0, 2)
        ).astype(ml_dtypes.float8_e4m3)
        bq_eff = beta @ wq_s + bq[hsel]
        bk_eff = beta @ wk_s + bk[hsel]
        bv_eff = beta @ wv_s + bv_full[hsel]
        bcol = np.concatenate([bq_eff, bk_eff]).reshape(4, 128).T.copy()
        # [64, 4, 512]: wo8[p, h, d] = Wo[64h+p, d] (DoubleRow head pairs)
        wo_s = np.ascontiguousarray(
            Wo[hsel, :].reshape(4, 64, D).transpose(1, 0, 2)
        ).astype(ml_dtypes.float8_e4m3)

        ln = int(lengths[n])
        b_idx = np.arange(128)[:, None]
        j_idx = np.arange(16)[None, :]
        pad01 = ((128 * j_idx + b_idx) < ln).astype(np.float32)

        resid = x[n] if c % 2 == 0 else np.ascontiguousarray(
            np.broadcast_to(bo, (S, D)).astype(np.float32)
        )

        in_maps.append(
            {
                "x": x[n],
                "resid": resid,
                "wqkv": wqkv,
                "bcol": np.ascontiguousarray(bcol, dtype=np.float32),
                "bv": bv_eff.astype(np.float32),
                "wo": wo_s,
                "pad01": np.ascontiguousarray(pad01, dtype=np.float32),
            }
        )
    return in_maps


def kernel_run(inputs, trace=False):
    from concourse.bass_utils import run_bass_kernel_spmd

    in_maps = _make_in_maps(inputs)
    bv_zero = all(
        not np.any(np.asarray(m["bv"], dtype=np.float32)) for m in in_maps
    )
    nc = _get_program(bv_zero)
    res = run_bass_kernel_spmd(nc, in_maps, list(range(N_CORES)), trace=trace)
    parts = [res.results[c]["y"] for c in range(N_CORES)]
    out = np.stack(
        [parts[2 * n] + parts[2 * n + 1] for n in range(4)], axis=0
    ).astype(np.float32)
    return out, res


def kernel(**inputs) -> np.ndarray:
    out, _ = kernel_run(inputs)
    return out



# revision 66
# speedup vs baseline: 1.2166x; 1.0012x over previous
"""Trainium2 Bass/Tile kernel for pre-LN causal multi-head self-attention.

Problem shapes (hardcoded): x (4, 2048, 512), 8 heads, dq=dv=64, fp32.

Sharding over 8 NeuronCores: core c handles batch n = c//2 and the 4 heads
h in [4*(c%2), 4*(c%2)+4).  Every core runs the SAME program (SPMD); all
per-core differences are carried by input values:
  - x:     the core's batch (2048, 512)
  - resid: even cores: x[n] (residual); odd cores: broadcast bo rows.
           Each term of (residual + bo) is added exactly once per pair.
  - wqkv:  (128, 4, 768) fp8e4 gamma-folded [Wq | Wk | Wv], g-slot packed:
           wqkv[p, g, c] = W[128g + p, c]
  - bcol:  (128, 4) q/k projection bias columns (beta @ W + b)
  - bv:    (256,) v-projection bias (zero-specialized when all-zero)
  - wo:    (64, 4, 512) fp8e4 Wo, wo[p, h, d] = Wo[64h + p, d]
  - pad01: (128, 16) key-padding mask column per key tile
Host combines: out[n] = y_part[2n] + y_part[2n+1].

Dataflow:
  LN:      bn_stats/bn_aggr on DVE; rstd = Exp(-0.5*Ln(var+eps)) on ACT
           (one table set); xn = (x-mu)*rstd in bf16 on GPSIMD.
  xnT8:    one [128,128] PE transpose per 128-d block -> psum bf16,
           evacuated (ACT/DVE alternating) with a cast to fp8:
           xnT8[p, g, s] = xn[s, 128g+p].
  q/k/v:   plain fp8 matmuls, one per g-slot (K=128 each; hw rejects
           DoubleRow chains mixing base partitions 0/64).  qT/kT f32r
           [128, S] (2 heads per tile on partition halves); v [s, 4, 65]
           f32r tiles with a ones column, padded key rows zeroed.
  scores:  sT[key, query] = kT^T @ qT (f32r, K=64), both heads into one
           2-bank psum tile -> ONE fused exp [128, 2, 512-o] per key tile
           (scale=1/8 folded in); diagonal tiles restrict to columns >= o
           and zero the sub-diagonal via GPSIMD affine_select.
  PV:      out'[65, 512] += v'^T @ p accumulated in psum; row 64 collects
           softmax denominators via the ones column.
  norm:    DVE reciprocal reads the denominator row straight from psum
           partition 64 into dre (f32r); a K=1 ones-matmul broadcasts it
           to partitions 0-63; one DVE multiply writes normalized fp8
           outT8 [64, 4(head), 2(ring), 512] from raw outSt.
  outproj: two fp8 DoubleRow matmuls (head pairs (0,1),(2,3), base 0) +
           in-place residual add on DVE; y DMA'd per chunk (per tile for
           the tail chunk).

Schedule: phase A (LN/transpose/projections) of chunk m+1 and the
normalize+outproj of chunk m-1 are queued as work items and popped into
block m's attention j-loop (deadline-checked), so PE interleaves them
into the exp-paced attention chain; x/resid/y move as per-chunk batched
DMAs (x chunk 0 split per-tile for a fast LN start).  One shared PSUM
pool budgets exactly 8 banks: pa(2) + s(2x2) + po0(1) + po1(1), with
rb->pa and psy->s tag reuse.  fp8 noise ~6-9e-3 rel vs the 2e-2 gate.
"""

import numpy as np

S = 2048
D = 512
DQ = 64
H_PER_CORE = 4
N_CORES = 8
LN_EPS = 1e-5
NEG = -30.0

_PROGRAM = {}


def _install_tile_patch():
    """Workarounds for walrus/concourse skew in this container:

    1. This walrus build rejects instructions carrying more than one
       semaphore-wait command ("Too many sync wait commands"), but Tile's
       rust wait-assigner freely emits 2-3 waits per instruction.  After
       wait assignment, split excess waits onto EventSemaphore carrier
       instructions inserted just before the owner on the same engine.
    2. Tile's tail drain carries one wait per outstanding proc; split into
       one drain per proc.
    3. Tile's tail emits a gpsimd sem_clear (Pool ISA opcode 176) that this
       walrus rejects ("ISA wrong length").  The NRT preamble's sema_reset
       zeroes user semaphores at every execution, so the tail clear is
       redundant and skipped.
    """
    from bass_rust import SyncInfo as _SyncInfo

    from concourse import mybir, tile
    from concourse.vector_clock import ScopedClock, VectorClock

    if getattr(tile.TileContext, "_mha_patch", False):
        return

    MAXW = 1

    def _split_excess_waits(ordered, nc):
        for bb_name, insts in list(ordered.items()):
            out = []
            changed = False
            for inst in insts:
                si = inst.sync_info
                if si is None:
                    out.append(inst)
                    continue
                waits = list(si.on_wait)
                if len(waits) > MAXW:
                    changed = True
                    excess = waits[:-MAXW]
                    for k in range(0, len(excess), MAXW):
                        carrier = mybir.InstEventSemaphore(
                            name=f"wsplit-{nc.next_id()}"
                        )
                        carrier.engine = inst.engine
                        carrier.bass_scheduled_proc = inst.bass_scheduled_proc
                        carrier.bass_scheduled_scope = inst.bass_scheduled_scope
                        carrier.bass_scheduled_tick = inst.bass_scheduled_tick
                        carrier.sync_info = _SyncInfo(
                            on_wait=excess[k : k + MAXW], on_update=[]
                        )
                        out.append(carrier)
                    si.on_wait = waits[-MAXW:]
                    inst.sync_info = si
                out.append(inst)
            if changed:
                ordered[bb_name] = out

    _RustTileClockWait = tile.TileClockWait

    class _SplittingTileClockWait:
        def __init__(self, tc, ordered, **kw):
            self._inner = _RustTileClockWait(tc, ordered, **kw)
            self._ordered = ordered
            self._nc = tc.nc

        def __getattr__(self, k):
            return getattr(self._inner, k)

        def assign_waits(self, bb_name):
            self._inner.assign_waits(bb_name)
            _split_excess_waits(self._ordered, self._nc)

    tile.TileClockWait = _SplittingTileClockWait

    def _patched_drain_and_barrier(self, tick_clock, wait_clock):
        gvec = tick_clock.global_clock
        n = len(gvec)
        for i in range(n):
            if gvec[i] > 0:
                v = VectorClock([gvec[j] if j == i else 0 for j in range(n)])
                d = self.nc.sync.drain()
                wait_clock.add_sem_waits(d.ins, ScopedClock({None: v}))
        self.nc.all_engine_barrier()
        assert self.sems is not None
        popped = self.nc._tile_sem_poison_stack.pop()
        assert popped is self._sem_poison
        self.nc.all_engine_barrier()

    tile.TileContext._drain_and_barrier = _patched_drain_and_barrier

    # cayman has 208 KiB usable per partition; the stale 192 KiB constant
    # leaves 16 KiB on the table
    from concourse import tile_utils

    tile_utils.max_sbuf_usage = 208 * 1024
    tile.TileContext._mha_patch = True


def _build_program(bv_zero=False):
    _BV_ZERO = bv_zero
    from contextlib import ExitStack

    import concourse.bass as bass
    import concourse.tile as tile
    from concourse import mybir
    from concourse.masks import make_identity

    f32 = mybir.dt.float32
    f32r = mybir.dt.float32r
    bf16 = mybir.dt.bfloat16
    fp8 = mybir.dt.float8e4
    DR = mybir.MatmulPerfMode.DoubleRow
    AF = mybir.ActivationFunctionType

    nc = bass.Bass()

    x_d = nc.dram_tensor("x", [S, D], f32, kind="ExternalInput")
    resid_d = nc.dram_tensor("resid", [S, D], f32, kind="ExternalInput")
    wqkv_d = nc.dram_tensor(
        "wqkv", [128, 4, 768], mybir.dt.float8e4, kind="ExternalInput"
    )
    bcol_d = nc.dram_tensor("bcol", [128, 4], f32, kind="ExternalInput")
    bv_d = nc.dram_tensor("bv", [256], f32, kind="ExternalInput")
    wo_d = nc.dram_tensor("wo", [64, 4, D], mybir.dt.float8e4, kind="ExternalInput")
    pad_d = nc.dram_tensor("pad01", [128, 16], f32, kind="ExternalInput")
    y_d = nc.dram_tensor("y", [S, D], f32, kind="ExternalOutput")

    _install_tile_patch()

    with tile.TileContext(nc) as tc, ExitStack() as ctx:
        consts = ctx.enter_context(tc.tile_pool(name="consts", bufs=1))
        big = ctx.enter_context(tc.tile_pool(name="big", bufs=1))
        # one shared PSUM pool; tag budget adds up to exactly 8 banks so all
        # phases can be in flight at once:
        #   pa(2) + sA(2) + sB(2) + poA(1) + poB(1) = 8
        # (rb reuses sA slots, psum_y reuses sB slots later in the kernel)
        ps = ctx.enter_context(tc.tile_pool(name="ps", bufs=2, space="PSUM"))
        xa_pool = ctx.enter_context(tc.tile_pool(name="xa", bufs=4))
        xn_pool = ctx.enter_context(tc.tile_pool(name="xn", bufs=8))
        st_pool = ctx.enter_context(tc.tile_pool(name="st", bufs=8))
        p_pool = ctx.enter_context(tc.tile_pool(name="pp", bufs=5))
        dt_pool = ctx.enter_context(tc.tile_pool(name="dt", bufs=5))
        r_pool = ctx.enter_context(tc.tile_pool(name="rr", bufs=2))

        # ---- constants and weights (x-chunk0 DMAs are issued FIRST in the
        # blk-0 body; all const DMAs are emitted after them so the LN start
        # isn't queued behind them on the single HWDGE ring) ----
        ident = consts.tile([128, 128], bf16, tag="ident")
        eps_col = consts.tile([128, 1], f32, tag="eps")
        nc.vector.memset(eps_col, LN_EPS)
        bcol = consts.tile([128, 4], f32, tag="bcol")
        pad01 = consts.tile([128, 16], f32, tag="pad01")
        wo8 = big.tile([64, 4, D], fp8, tag="wo8", name="wo8")
        ones64 = consts.tile([65, 64], f32r, tag="ones64")
        nc.gpsimd.memset(ones64.bitcast(f32)[64:65, :], 1.0)
        if not _BV_ZERO:
            bv_bc = consts.tile([128, 256], f32, tag="bv")

        # per-chunk batched views: row s = c*512 + a*128 + p
        x_r = x_d.rearrange("(c a p) d -> c p a d", a=4, p=128)
        resid_r = resid_d.rearrange("(c a p) d -> c p a d", a=4, p=128)
        y_r = y_d.rearrange("(c a p) d -> c p a d", a=4, p=128)
        wqkv_sb = big.tile([128, 4, 768], fp8, tag="wqkv8", name="wqkv8")

        # persistent big tensors.  xnT8[p, g, s] = xn[s, 128g + p] in fp8:
        # each g-slot is one contiguous K=128 contraction block for the
        # plain-fp8 projection matmuls, written by one [128,128] PE
        # transpose per slot and evacuated at full 128-partition width.
        xnT8 = big.tile([128, 4, S], fp8, tag="xnT8", name="xnT8")
        qT = [big.tile([128, S], f32r, tag=f"qT{hp}", name=f"qT{hp}") for hp in range(2)]
        kT = [big.tile([128, S], f32r, tag=f"kT{hp}", name=f"kT{hp}") for hp in range(2)]
        v_sb = [big.tile([128, 4, 65], f32r, tag=f"v{st}", name=f"vsb{st}") for st in range(16)]
        # attention outputs, head h on partitions 0-63, slot h: outSt holds
        # raw numerators (2-chunk ring, consumed by the normalize drip one
        # block later), outT8 the normalized fp8 (2-chunk ring) feeding the
        # DoubleRow output projection.
        outSt = big.tile([64, 4, 2, 512], f32, tag="outSt", name="outSt")
        outT8 = big.tile([64, 4, 2, 512], fp8, tag="outT8", name="outT8")

        # ==== interleaved main loop: chunk blk of LN/proj, then the two
        # head-pairs' attention m=blk blocks (which need only chunks <= blk).
        # This keeps ScalarE (exp-bound) fed while PE does projections. ====
        from collections import deque

        pending_work = deque()
        x4_tiles = {}
        x0_tiles = []
        r4_tiles = {}
        dre_tiles = {}
        xn_map = {}

        def phaseA_ln(chv, ss):
            x_t = x0_tiles[ss] if chv == 0 else x4_tiles[chv][:, ss, :]
            stats = st_pool.tile([128, 6], f32, tag="stats", name="stats")
            nc.vector.bn_stats(out=stats, in_=x_t)
            mv = st_pool.tile([128, 2], f32, tag="mv", name="mv")
            nc.vector.bn_aggr(out=mv, in_=stats)
            lnv = st_pool.tile([128, 1], f32, tag="lnv", name="lnv")
            nc.scalar.activation(
                out=lnv, in_=mv[:, 1:2], func=AF.Ln, bias=eps_col, scale=1.0
            )
            rstd = st_pool.tile([128, 1], f32, tag="rstd", name="rstd")
            nc.scalar.activation(
                out=rstd, in_=lnv, func=AF.Exp, bias=0.0, scale=-0.5
            )
            xn_t = xn_pool.tile([128, D], bf16, tag="xn", name="xnt")
            # on Pool (gpsimd): all-SBUF op, and Pool is otherwise idle
            nc.gpsimd.tensor_scalar(
                out=xn_t,
                in0=x_t,
                scalar1=mv[:, 0:1],
                scalar2=rstd,
                op0=mybir.AluOpType.subtract,
                op1=mybir.AluOpType.mult,
            )
            xn_map[(chv, ss)] = xn_t

        def phaseA_tr(chv, ss):
            # transpose [128, 64] blocks: xn [s,d] -> xnT8 [64e+p, g, s].
            # Transposes run in bf16 (hw rejects fp8 transpose without
            # stride-2 packing); the evacuation copy casts to fp8, which
            # is what the DoubleRow projections consume.
            stv = chv * 4 + ss
            xn_t = xn_map.pop((chv, ss))
            ps_t = ps.tile([128, 4, 128], bf16, tag="pa", name="pst")
            # one [128,128] transpose per g-slot: out partition p = 64e+p64
            # lands d = 128g + 64e + p64, exactly the DoubleRow packing
            for g in range(4):
                nc.tensor.transpose(
                    out=ps_t[:, g, :],
                    in_=xn_t[:, g * 128 : (g + 1) * 128],
                    identity=ident,
                )
            if ss % 2 == 0:
                nc.scalar.copy(
                    out=xnT8[:, :, stv * 128 : (stv + 1) * 128], in_=ps_t
                )
            else:
                nc.vector.tensor_copy(
                    out=xnT8[:, :, stv * 128 : (stv + 1) * 128], in_=ps_t
                )

        DRPAIRS = [(0, 0), (0, 1), (1, 0), (1, 1)]

        def phaseA_qk(chv, jt):
            # q/k projection j-tile (q-pair0, q-pair1, k-pair0, k-pair1).
            # Plain fp8 matmuls: each g-slot covers contiguous d range
            # [128g, 128g+128) across the full 128 partitions (hw rejects
            # DoubleRow chains that mix base partitions 0/64).
            dst = qT[jt] if jt < 2 else kT[jt - 2]
            ps_qk = ps.tile([128, 512], f32, tag="pa", name="psqk")
            for g in range(4):
                nc.tensor.matmul(
                    out=ps_qk,
                    lhsT=wqkv_sb[:, g, jt * 128 : (jt + 1) * 128],
                    rhs=xnT8[:, g, chv * 512 : (chv + 1) * 512],
                    start=(g == 0),
                    stop=(g == 3),
                )
            nc.vector.tensor_scalar_add(
                out=dst[:, chv * 512 : (chv + 1) * 512],
                in0=ps_qk,
                scalar1=bcol[:, jt : jt + 1],
            )

        def phaseA_v(chv, ss):
            # v projection: [s, e] orientation with ones column + padding
            stv = chv * 4 + ss
            ps_v = ps.tile([128, 256], f32, tag="pa", name="psv")
            for g in range(4):
                nc.tensor.matmul(
                    out=ps_v,
                    lhsT=xnT8[:, g, stv * 128 : (stv + 1) * 128],
                    rhs=wqkv_sb[:, g, 512:768],
                    start=(g == 0),
                    stop=(g == 3),
                )
            vt = v_sb[stv]
            nc.gpsimd.memset(vt.bitcast(f32)[:, :, 64:65], 1.0)
            if _BV_ZERO:
                # (psum + 0) * pad in one op; the ones column is scaled
                # by a separate tiny op
                nc.vector.tensor_scalar_mul(
                    out=vt[:, :, 0:64],
                    in0=ps_v.rearrange("p (h e) -> p h e", h=4),
                    scalar1=pad01[:, stv : stv + 1],
                )
                nc.vector.tensor_scalar_mul(
                    out=vt[:, :, 64:65],
                    in0=vt[:, :, 64:65],
                    scalar1=pad01[:, stv : stv + 1],
                )
            else:
                nc.vector.tensor_tensor(
                    out=vt[:, :, 0:64],
                    in0=ps_v.rearrange("p (h e) -> p h e", h=4),
                    in1=bv_bc.rearrange("p (h e) -> p h e", h=4),
                    op=mybir.AluOpType.add,
                )
                nc.vector.tensor_scalar_mul(
                    out=vt[:, :, :],
                    in0=vt[:, :, :],
                    scalar1=pad01[:, stv : stv + 1],
                )

        def norm_row(h, mm):
            # rb = K=1 ones-matmul broadcast of the reciprocal row (psum
            # partition 64 -> partitions 0-63), then one DVE multiply
            # raw numerators -> normalized fp8 (DoubleRow outproj layout)
            hq, abq = divmod(h, 2)
            rb = ps.tile([64, 512], f32, tag="pa", name="rb")
            nc.tensor.matmul(
                out=rb,
                lhsT=ones64[64:65, :],
                rhs=dre_tiles[(hq, mm)][64:65, abq, :],
                start=True,
                stop=True,
            )
            nc.vector.tensor_tensor(
                out=outT8[:, h, mm % 2, :],
                in0=outSt[:, h, mm % 2, :],
                in1=rb,
                op=mybir.AluOpType.mult,
            )

        def load_resid(mm):
            r4 = r_pool.tile([128, 4, D], f32, tag="r4", name="r4")
            nc.sync.dma_start(out=r4, in_=resid_r[mm])
            r4_tiles[mm] = r4

        def outproj_tile(stv):
            mm, ssl = divmod(stv, 4)
            r4 = r4_tiles[mm]
            ps_y = ps.tile([128, 512], f32, tag="s", name="psy")
            for gp in range(2):
                nc.tensor.matmul(
                    out=ps_y,
                    lhsT=outT8[
                        :, 2 * gp : 2 * gp + 2, mm % 2,
                        ssl * 128 : (ssl + 1) * 128,
                    ],
                    rhs=wo8[:, 2 * gp : 2 * gp + 2, :],
                    start=(gp == 0),
                    stop=(gp == 1),
                    perf_mode=DR,
                )
            # y written in place over the residual slice; one DMA per
            # chunk (tail chunk: per-tile DMAs for latency)
            nc.vector.tensor_tensor(
                out=r4[:, ssl, :], in0=ps_y, in1=r4[:, ssl, :],
                op=mybir.AluOpType.add,
            )
            if mm == 3:
                nc.sync.dma_start(
                    out=y_d[stv * 128 : (stv + 1) * 128, :],
                    in_=r4[:, ssl, :],
                )
            elif ssl == 3:
                nc.sync.dma_start(out=y_r[mm], in_=r4)

        for blk in range(4):
            chv = blk
            # ---- phase A: chunk 0 runs inline before any attention; later
            # chunks are queued as pieces and popped during the PREVIOUS
            # block's attention j-loop so PE interleaves projection work
            # into the exp-gated attention chain ----
            if chv == 0:
                # chunk 0: four separate x DMAs so ss=0's LN starts as soon
                # as its 128 rows land; consts/weights queue behind them
                for ss in range(4):
                    x_t = xa_pool.tile([128, D], f32, tag="x", name="xt")
                    nc.sync.dma_start(
                        out=x_t, in_=x_d[ss * 128 : (ss + 1) * 128, :]
                    )
                    x0_tiles.append(x_t)
                make_identity(nc, ident)
                nc.sync.dma_start(out=wqkv_sb, in_=wqkv_d[:, :, :])
                nc.sync.dma_start(out=bcol, in_=bcol_d[:, :])
                nc.sync.dma_start(out=pad01, in_=pad_d[:, :])
                if not _BV_ZERO:
                    nc.sync.dma_start(
                        out=bv_bc, in_=bv_d[None, :].to_broadcast([128, 256])
                    )
                for ss in range(4):
                    phaseA_ln(0, ss)
                    phaseA_tr(0, ss)
                for jt in range(4):
                    phaseA_qk(0, jt)
                for ss in range(4):
                    phaseA_v(0, ss)
                # late-loaded weights (needed at outproj)
                nc.sync.dma_start(out=wo8, in_=wo_d[:, :, :])
            else:
                # safety: everything chunk-blk must be emitted by now
                while pending_work and pending_work[0][2] <= blk:
                    fn, args, _ = pending_work.popleft()
                    fn(*args)
            if chv < 3:
                # prefetch next chunk's x as one batched DMA, then queue its
                # phase A pieces (deadline: before block chv+1's attention)
                nxt = xa_pool.tile([128, 4, D], f32, tag="x4", bufs=2, name="x4")
                nc.sync.dma_start(out=nxt, in_=x_r[chv + 1])
                x4_tiles[chv + 1] = nxt
                d = chv + 1
                for ss in range(4):
                    pending_work.append((phaseA_ln, (d, ss), d))
                    pending_work.append((phaseA_tr, (d, ss), d))
                for jt in range(4):
                    pending_work.append((phaseA_qk, (d, jt), d))
                for ss in range(4):
                    pending_work.append((phaseA_v, (d, ss), d))

            # ---- attention m = blk for both head-pairs ----
            m = blk
            for hp in range(2):
                po = {}
                for ab in range(2):
                    po[ab] = ps.tile(
                        [65, 512], f32, tag=f"po{ab}", bufs=1, name=f"po{ab}"
                    )
                njt = 4 * m + 4  # key tiles 0 .. 4m+3
                for j in range(njt):
                    diag_o = 128 * (j - 4 * m) if j >= 4 * m else None
                    o = diag_o if diag_o is not None else 0
                    # both heads' scores in one 2-bank tile -> one fused exp;
                    # diagonal tiles restrict to query columns >= o (the
                    # sub-diagonal columns are never read downstream)
                    ps_s = ps.tile([128, 2, 512], f32, tag="s", name="pss")
                    for ab in range(2):
                        base = ab * 64
                        nc.tensor.matmul(
                            out=ps_s[:, ab, o:512],
                            lhsT=kT[hp][base : base + 64, j * 128 : (j + 1) * 128],
                            rhs=qT[hp][
                                base : base + 64, m * 512 + o : (m + 1) * 512
                            ],
                            start=True,
                            stop=True,
                        )
                    p_t = p_pool.tile([128, 2, 512], f32r, tag="p", name="pt")
                    nc.scalar.activation(
                        out=p_t[:, :, o:512],
                        in_=ps_s[:, :, o:512],
                        func=AF.Exp,
                        bias=0.0,
                        scale=0.125,
                    )
                    if diag_o is not None:
                        # zero strictly-below-diagonal: keep col-part>=0
                        nc.gpsimd.affine_select(
                            out=p_t[:, :, o : o + 128],
                            in_=p_t[:, :, o : o + 128],
                            compare_op=mybir.AluOpType.is_ge,
                            fill=0.0,
                            base=0,
                            pattern=[[0, 2], [1, 128]],
                            channel_multiplier=-1,
                        )
                    for ab in range(2):
                        h = hp * 2 + ab
                        nc.tensor.matmul(
                            out=po[ab][:, o:512],
                            lhsT=v_sb[j][:, h, 0:65],
                            rhs=p_t[:, ab, o:512],
                            start=(j == 0),
                            stop=(j == njt - 1),
                        )
                    # blk 0's short loop needs 2 pops/j to cover chunk 1's
                    # 16 phase-A pieces; later blocks pace at 1/j.  blk 3
                    # halves the rate again so the mm=2 drips also fill
                    # the second head-pair's exp-paced PE gaps.
                    npop = 2 if blk == 0 else (1 if blk < 3 else (j + hp) % 2)
                    for _ in range(npop):
                        if pending_work:
                            fn, args, _d = pending_work.popleft()
                            fn(*args)
                # reciprocal the denominator rows straight out of psum
                # partition 64 into dre (broadcast later by a K=1 ones
                # matmul in the normalize drip), then evacuate the raw
                # numerators to outSt (ring slot m%2); recips first so the
                # m=3 tail's normalize chain starts as early as possible
                dre = dt_pool.tile([65, 2, 512], f32r, tag="dre", name="dre")
                with nc.allow_low_precision(reason="f32r recip row for PE broadcast"):
                    for ab in range(2):
                        nc.vector.reciprocal(
                            out=dre[64:65, ab, :],
                            in_=po[ab][64:65, :],
                        )
                dre_tiles[(hp, m)] = dre
                for ab in range(2):
                    if blk == 3:
                        # tail: ACT is idle here, and this shortens the
                        # DVE chain gating the last outproj
                        nc.scalar.copy(
                            out=outSt[:, hp * 2 + ab, m % 2, :],
                            in_=po[ab][0:64, :],
                        )
                    else:
                        nc.vector.tensor_copy(
                            out=outSt[:, hp * 2 + ab, m % 2, :],
                            in_=po[ab][0:64, :],
                        )

                if blk == 3:
                    # m=3 normalize immediately (short tail); both rb
                    # broadcasts first so the DVE multiplies pipeline
                    rbs = {}
                    for ab in range(2):
                        rb = ps.tile([64, 512], f32, tag="pa", name="rb")
                        nc.tensor.matmul(
                            out=rb,
                            lhsT=ones64[64:65, :],
                            rhs=dre[64:65, ab, :],
                            start=True,
                            stop=True,
                        )
                        rbs[ab] = rb
                    for ab in range(2):
                        h = hp * 2 + ab
                        nc.vector.tensor_tensor(
                            out=outT8[:, h, 1, :],
                            in0=outSt[:, h, 1, :],
                            in1=rbs[ab],
                            op=mybir.AluOpType.mult,
                        )
                    if hp == 1:
                        while pending_work:
                            fn, args, _d = pending_work.popleft()
                            fn(*args)
                        for ss in range(4):
                            outproj_tile(12 + ss)

            # ---- queue chunk-m normalize + output projection; popped
            # during block m+1's attention j-loop ----
            if blk < 3:
                d = blk + 2
                for h_ in range(4):
                    pending_work.append((norm_row, (h_, blk), d))
                pending_work.append((load_resid, (blk,), d))
                for s_ in range(4):
                    pending_work.append((outproj_tile, (blk * 4 + s_,), d))
                if blk == 2:
                    pending_work.append((load_resid, (3,), 4))

    return nc


def _get_program(bv_zero=False):
    if bv_zero not in _PROGRAM:
        _PROGRAM[bv_zero] = _build_program(bv_zero)
    return _PROGRAM[bv_zero]


def _make_in_maps(inputs):
    x = np.ascontiguousarray(np.asarray(inputs["x"], dtype=np.float32))
    lengths = np.asarray(inputs["key_value_sequence_lengths"]).astype(np.int64)
    Wq = np.asarray(inputs["Wq"], dtype=np.float32)
    bq = np.asarray(inputs["bq"], dtype=np.float32)
    Wkv = np.asarray(inputs["Wkv"], dtype=np.float32)
    bkv = np.asarray(inputs["bkv"], dtype=np.float32)
    Wo = np.asarray(inputs["Wo"], dtype=np.float32)
    bo = np.asarray(inputs["bo"], dtype=np.float32)
    gamma = np.asarray(inputs["gamma"], dtype=np.float32)
    beta = np.asarray(inputs["beta"], dtype=np.float32)

    H = 8
    Wk = Wkv[:, : H * DQ]
    Wv = Wkv[:, H * DQ :]
    bk = bkv[: H * DQ]
    bv_full = bkv[H * DQ :]

    in_maps = []
    for c in range(N_CORES):
        n = c // 2
        h0 = 4 * (c % 2)
        hsel = slice(h0 * DQ, (h0 + 4) * DQ)  # 256 contiguous columns

        wq_s = Wq[:, hsel]
        wk_s = Wk[:, hsel]
        wv_s = Wv[:, hsel]
        import ml_dtypes

        wqkv = np.concatenate(
            [gamma[:, None] * wq_s, gamma[:, None] * wk_s, gamma[:, None] * wv_s],
            axis=1,
        )
        # [128, 4, 768]: wqkv8[p, g, c] = wqkv[128g + p, c] (g-slot = one
        # contiguous K=128 contraction block)
        wqkv = np.ascontiguousarray(
            wqkv.reshape(4, 128, 768).transpose(1, 0, 2)
        ).astype(ml_dtypes.float8_e4m3)
        bq_eff = beta @ wq_s + bq[hsel]
        bk_eff = beta @ wk_s + bk[hsel]
        bv_eff = beta @ wv_s + bv_full[hsel]
        bcol = np.concatenate([bq_eff, bk_eff]).reshape(4, 128).T.copy()
        # [64, 4, 512]: wo8[p, h, d] = Wo[64h+p, d] (DoubleRow head pairs)
        wo_s = np.ascontiguousarray(
            Wo[hsel, :].reshape(4, 64, D).transpose(1, 0, 2)
        ).astype(ml_dtypes.float8_e4m3)

        ln = int(lengths[n])
        b_idx = np.arange(128)[:, None]
        j_idx = np.arange(16)[None, :]
        pad01 = ((128 * j_idx + b_idx) < ln).astype(np.float32)

        resid = x[n] if c % 2 == 0 else np.ascontiguousarray(
            np.broadcast_to(bo, (S, D)).astype(np.float32)
        )

        in_maps.append(
            {
                "x": x[n],
                "resid": resid,
                "wqkv": wqkv,
                "bcol": np.ascontiguousarray(bcol, dtype=np.float32),
                "bv": bv_eff.astype(np.float32),
                "wo": wo_s,
                "pad01": np.ascontiguousarray(pad01, dtype=np.float32),
            }
        )
    return in_maps


def kernel_run(inputs, trace=False):
    from concourse.bass_utils import run_bass_kernel_spmd

    in_maps = _make_in_maps(inputs)
    bv_zero = all(
        not np.any(np.asarray(m["bv"], dtype=np.float32)) for m in in_maps
    )
    nc = _get_program(bv_zero)
    res = run_bass_kernel_spmd(nc, in_maps, list(range(N_CORES)), trace=trace)
    parts = [res.results[c]["y"] for c in range(N_CORES)]
    out = np.stack(
        [parts[2 * n] + parts[2 * n + 1] for n in range(4)], axis=0
    ).astype(np.float32)
    return out, res


def kernel(**inputs) -> np.ndarray:
    out, _ = kernel_run(inputs)
    return out

